# revision 1
# baseline (speedup 1.0000x reference)
"""ConditionGateAttention Trainium2 kernel.

Gated dual-attention block: causal self-attention + cross-attention to a
77-token condition, sigmoid cross-gating, output projection.

  B=2, T=2048, M=77, C=512, H=8 heads, D=64.

Sharding (8 cores): batch x sequence. Core = (b=core//4, j=core%4); each
core owns query chunks {j, 7-j} of 256 rows of its batch (balanced causal
work) and computes K/V for the full batch locally (no collectives).

Layouts: activations kept transposed ([C, tokens]) end-to-end so every
matmul consumes the previous one's output directly (zero on-chip
transposes). Matmul inputs fp16 (full PE rate), fp32 PSUM accumulate.

Masking: the host inspects attn_mask and derives, per query-chunk
position, a program-uniform k-extent (max over cores) plus per-core 0/1
mask tiles multiplied onto the exp output on DVE (one [128,1024] multiply
per masked k-group; SPMD-uniform program, per-core data). padding_mask
becomes a per-partition ACT bias on the cross-attention exp. Softmax
denominators come free from a ones-column appended to V; normalization is
reciprocal + DMA-broadcast + DVE mult. A lag-2 software pipeline keeps PE
busy across the exp latency (QK(g+2) issues before AV(g)).
"""
import numpy as np
import ml_dtypes
from contextlib import ExitStack

import concourse.bass as bass
import concourse.tile as tile
from concourse import bacc, mybir
from concourse import bass_utils

B, T, M, C, H = 2, 2048, 77, 512, 8
D = C // H            # 64
P = 128
KI = C // P           # 4 contraction chunks
PAIRS = H // 2        # 4 head pairs (pair i = heads 2i, 2i+1 = C rows 128i..128i+128)
QC = 256              # query chunk size (2 chunks per core)
NCHUNK = T // QC      # 8 chunks per batch
TQ = 2 * QC           # local queries per core
KT = 128              # k-tile size (partition dim of transposed logits)
GROUP = 4             # k-tiles per logits psum group ([128, 1024] fp32 = 2 banks)
NEG = -30000.0        # mask bias (fp16-representable; exp(-30000+s) == 0 in fp32)
MP = 128              # condition length M=77 zero-padded to 128 on host
DA = D + 1            # V augmented with a ones-column (denominator row)

f16 = mybir.dt.float16
f32 = mybir.dt.float32
AF = mybir.ActivationFunctionType
ALU = mybir.AluOpType

_cache = {}


def _chunks_of_core(j):
    return (j, NCHUNK - 1 - j)


def build_program(ext, bias_slots, has_b, stage=4, repeat=1):
    """ext: per-position k-extent in KT tiles (uniform across cores), rounded
    up to GROUP. bias_slots: list of (pos, slot) needing a bias tile (uniform;
    data per-core). has_b: dict of which projection biases are nonzero.
    stage: 0=io-baseline, 1=projections, 15=kc, 2=+self-attn, 31/32/3=+cross
    pieces, 4=full. repeat: run the compute body N times (timing aid)."""
    key = (tuple(ext), tuple(bias_slots), tuple(sorted(has_b.items())),
           stage, repeat)
    if key in _cache:
        return _cache[key]

    nb = len(bias_slots)
    bias_idx = {ps: n for n, ps in enumerate(bias_slots)}

    nc = bacc.Bacc("TRN2", num_devices=8, debug=False)

    xT_d = nc.dram_tensor("xT", [C, T], f16, kind="ExternalInput").ap()
    xqT_d = nc.dram_tensor("xqT", [C, TQ], f16, kind="ExternalInput").ap()
    cT_d = nc.dram_tensor("cT", [C, MP], f16, kind="ExternalInput").ap()
    w_d = {n: nc.dram_tensor(f"w{n}", [C, C], f16, kind="ExternalInput").ap()
           for n in ["q", "k", "v", "kc", "vc", "g1", "g2", "p"]}
    ident_d = nc.dram_tensor("ident", [P, P], f16, kind="ExternalInput").ap()
    pad_d = nc.dram_tensor("padb", [P, 1], f32, kind="ExternalInput").ap()
    if nb:
        bias_d = nc.dram_tensor("biasm", [nb, P, GROUP * QC], f16, kind="ExternalInput").ap()
    bv_d = {}
    for n in ["q", "k", "kc", "g1", "g2"]:
        if has_b[n]:
            bv_d[n] = nc.dram_tensor(f"b{n}", [P, KI], f32, kind="ExternalInput").ap()
    for n in ["v", "vc", "p"]:
        if has_b[n]:
            bv_d[n] = nc.dram_tensor(f"b{n}", [1, C], f16, kind="ExternalInput").ap()
    out_d = nc.dram_tensor("out", [TQ, C], f32, kind="ExternalOutput").ap()

    def emit(tc, ctx):
        consts = ctx.enter_context(tc.tile_pool(name="consts", bufs=1))
        acts = ctx.enter_context(tc.tile_pool(name="acts", bufs=1))
        work = ctx.enter_context(tc.tile_pool(name="work", bufs=4))
        nrm = ctx.enter_context(tc.tile_pool(name="nrm", bufs=4))
        ps_a = ctx.enter_context(tc.tile_pool(name="ps_a", bufs=2, space="PSUM"))
        ps_b = ctx.enter_context(tc.tile_pool(name="ps_b", bufs=2, space="PSUM"))
        ps_y = ctx.enter_context(tc.tile_pool(name="ps_y", bufs=2, space="PSUM"))

        # ---- load constants/inputs ----
        def chunked(ap):  # [C, n] dram -> [128, 4, n] view
            return ap.rearrange("(o p) n -> p o n", p=P)

        # DMA order matters: the q-projection inputs land first so PE can
        # start while the bulk (xT, V/gate weights, bias tiles) streams in.
        w_sb = {n: consts.tile([P, KI, C], f16, name=f"w{n}") for n in w_d}
        xqT_sb = consts.tile([P, KI, TQ], f16, name="xqT")
        nc.sync.dma_start(xqT_sb[:], chunked(xqT_d))
        nc.sync.dma_start(w_sb["q"][:], chunked(w_d["q"]))
        xT_sb = consts.tile([P, KI, T], f16, name="xT")
        nc.sync.dma_start(xT_sb[:], chunked(xT_d))
        nc.sync.dma_start(w_sb["k"][:], chunked(w_d["k"]))
        cT_sb = consts.tile([P, KI, MP], f16, name="cT")
        nc.sync.dma_start(cT_sb[:], chunked(cT_d))
        for n in ["kc", "v", "vc", "g1", "g2", "p"]:
            nc.sync.dma_start(w_sb[n][:], chunked(w_d[n]))
        ident = consts.tile([P, P], f16, name="ident")
        nc.sync.dma_start(ident[:], ident_d)
        pad_sb = consts.tile([P, 1], f32, name="padb")
        nc.sync.dma_start(pad_sb[:], pad_d)
        if nb:
            bias_sb = consts.tile([P, nb, GROUP * QC], f16, name="biasm")
            nc.sync.dma_start(bias_sb[:], bias_d.rearrange("n p q -> p n q"))
        bv_sb = {}
        for n, d in bv_d.items():
            if n in ("v", "vc", "p"):
                bv_sb[n] = consts.tile([P, C], f16, name=f"b{n}")
                nc.sync.dma_start(bv_sb[n][:],
                                  d[0:1, :].unsqueeze(1).to_broadcast((1, P, C)))
            else:
                bv_sb[n] = consts.tile([P, KI], f32, name=f"b{n}")
                nc.sync.dma_start(bv_sb[n][:], d)

        def dump(srcs):
            for m, src in enumerate(srcs):
                osb = work.tile([P, C], f32, tag="osb")
                w = src.shape[-1]
                if w < C:
                    nc.vector.memset(osb[:], 0.0)
                nc.vector.tensor_copy(osb[:, 0:w], src)
                nc.sync.dma_start(out_d[P * m:P * m + P, :], osb[:])

        if stage == 0:
            # IO-only baseline: same inputs/outputs, no compute
            for m in range(PAIRS):
                osb = work.tile([P, C], f32, tag="osb")
                nc.vector.memset(osb[:], 0.0)
                nc.sync.dma_start(out_d[P * m:P * m + P, :], osb[:])
            return

        # ---- persistent activation tiles ----
        qT_sb = [acts.tile([P, TQ], f16, name=f"qT{i}") for i in range(PAIRS)]
        kT_sb = [acts.tile([P, T], f16, name=f"kT{i}") for i in range(PAIRS)]
        kcT_sb = [acts.tile([P, MP], f16, name=f"kcT{i}") for i in range(PAIRS)]
        v_sb = [acts.tile([P, H * DA], f16, name=f"v{m}") for m in range(T // P)]
        vc_sb = [acts.tile([P, H * DA], f16, name="vc")]
        yT_sb = [acts.tile([P, TQ], f16, name=f"yT{i}") for i in range(PAIRS)]
        ycT_sb = [acts.tile([P, TQ], f16, name=f"ycT{i}") for i in range(PAIRS)]
        g1_sb = [acts.tile([P, TQ], f16, name=f"g1_{o}") for o in range(PAIRS)]
        g2_sb = [acts.tile([P, TQ], f16, name=f"g2_{o}") for o in range(PAIRS)]
        z_sb = [acts.tile([P, TQ], f16, name=f"z{o}") for o in range(PAIRS)]

        # ---- projections ----
        def proj_T(wname, rhs_sb, n_free, out_tiles, free_tile):
            # out[Cout, n] = W.T @ actT ; out_tiles[i] [128, n_free] f16
            for i in range(PAIRS):
                for tt in range(0, n_free, free_tile):
                    fw = min(free_tile, n_free - tt)
                    ps = ps_b.tile([P, 512], f32, tag="psb")
                    for ki in range(KI):
                        nc.tensor.matmul(ps[:, 0:fw],
                                         w_sb[wname][:, ki, P * i:P * i + P],
                                         rhs_sb[:, ki, tt:tt + fw],
                                         start=(ki == 0), stop=(ki == KI - 1))
                    if has_b[wname]:
                        nc.scalar.activation(out_tiles[i][:, tt:tt + fw], ps[:, 0:fw],
                                             AF.Identity, bias=bv_sb[wname][:, i:i + 1])
                    else:
                        nc.vector.tensor_copy(out_tiles[i][:, tt:tt + fw], ps[:, 0:fw])

        # V in natural layout, ones-augmented per head: [tok, H*(D+1)]
        def vproj(wname, src_sb, rows, row_tiles, out_tiles, ones_rows=None):
            for m in range(row_tiles):
                pr = min(P, rows - m * P)
                ones_r = pr if ones_rows is None else min(ones_rows, pr)
                ps = ps_b.tile([P, 512], f32, tag="psb")
                if pr < P:
                    nc.vector.memset(out_tiles[m][:], 0.0)
                for ki in range(KI):
                    nc.tensor.matmul(ps[0:pr, :],
                                     src_sb[:, ki, m * P:m * P + pr],
                                     w_sb[wname][:, ki, :],
                                     start=(ki == 0), stop=(ki == KI - 1))
                dst = out_tiles[m].rearrange("p (h e) -> p h e", e=DA)
                nc.vector.tensor_copy(dst[0:pr, :, 0:D],
                                      ps[0:pr, :].rearrange("p (h e) -> p h e", e=D))
                if has_b[wname]:
                    nc.vector.tensor_tensor(
                        dst[0:pr, :, 0:D], dst[0:pr, :, 0:D],
                        bv_sb[wname][0:pr, :].rearrange("p (h e) -> p h e", e=D),
                        ALU.add)
                if ones_r < pr:
                    nc.vector.memset(dst[:, :, D:DA], 0.0)
                nc.vector.memset(dst[0:ones_r, :, D:DA], 1.0)

        def projections():
            proj_T("q", xqT_sb, TQ, qT_sb, 512)
            proj_T("k", xT_sb, T, kT_sb, 512)
            # c zero-padded to MP=128 tokens on host -> 128-clean cross
            # shapes; padded K_c/V_c columns are zero, junk logit rows see
            # exp(0)=1 but multiply against zero V_c rows + zero ones-col.
            proj_T("kc", cT_sb, MP, kcT_sb, MP)
            vproj("v", xT_sb, T, T // P, v_sb)
            vproj("vc", cT_sb, MP, 1, vc_sb, ones_rows=M)

        # ---- attention ----
        def attention():
            for i in range(PAIRS):
                for pos in range(2):
                    q0 = pos * QC
                    ngrp = ext[pos] // GROUP
                    yps = ps_y.tile([DA, 2 * QC], f32, tag="y")
                    for hb in range(2):  # heads 2i (rows 0:64) / 2i+1 (64:128)
                        b0 = hb * D
                        yslice = yps[:, hb * QC:(hb + 1) * QC]

                        def qk_group(g):
                            # logits for k-tiles [4g, 4g+4); masking is a 0/1
                            # multiply on the exp output, run on the otherwise
                            # idle GPSIMD engine (PE does pure QK matmuls)
                            lg = ps_a.tile([P, GROUP * QC], f32, tag="lg")
                            for s4 in range(GROUP):
                                s = g * GROUP + s4
                                nc.tensor.matmul(
                                    lg[:, s4 * QC:(s4 + 1) * QC],
                                    kT_sb[i][b0:b0 + D, s * KT:(s + 1) * KT],
                                    qT_sb[i][b0:b0 + D, q0:q0 + QC],
                                    start=True, stop=True)
                            pt = work.tile([P, GROUP * QC], f16, tag="pt")
                            nc.scalar.activation(pt[:], lg[:], AF.Exp)
                            if (pos, g) in bias_idx:
                                nc.vector.tensor_tensor(
                                    pt[:], pt[:],
                                    bias_sb[:, bias_idx[(pos, g)], :], ALU.mult)
                            return pt

                        def av_group(g, pt):
                            for s4 in range(GROUP):
                                s = g * GROUP + s4
                                nc.tensor.matmul(
                                    yslice,
                                    v_sb[s][:, (2 * i + hb) * DA:(2 * i + hb + 1) * DA],
                                    pt[:, s4 * QC:(s4 + 1) * QC],
                                    start=(s == 0), stop=(s == ext[pos] - 1))

                        # lag-2 software pipeline: QK(g+2) is emitted before
                        # AV(g) so the exp(g) latency hides behind PE work
                        pts = {}
                        for g in range(ngrp):
                            pts[g] = qk_group(g)
                            if g >= 2:
                                av_group(g - 2, pts.pop(g - 2))
                        for g in range(max(0, ngrp - 2), ngrp):
                            av_group(g, pts.pop(g))
                    # cross-attention for this (pair, chunk)
                    branches = [(yps, yT_sb)]
                    do_cqk = stage in (31, 32, 3, 4)
                    do_avc = stage in (32, 3, 4)
                    do_cnorm = stage in (3, 4)
                    if do_cqk:
                        # separate PSUM tiles (banks) per head: the two S_c
                        # matmuls run concurrently in different PE row-groups
                        # and must not drain into the same PSUM bank.
                        scp = [ps_b.tile([P, QC], f32, tag="psb", name=f"scp{hb}")
                               for hb in range(2)]
                        pct = work.tile([P, 2 * QC], f16, tag="pct")
                        for hb in range(2):
                            b0 = hb * D
                            nc.tensor.matmul(scp[hb][:, 0:QC],
                                             kcT_sb[i][b0:b0 + D, :],
                                             qT_sb[i][b0:b0 + D, q0:q0 + QC],
                                             start=True, stop=True)
                            nc.scalar.activation(pct[:, hb * QC:(hb + 1) * QC],
                                                 scp[hb][:, 0:QC], AF.Exp,
                                                 bias=pad_sb[:, 0:1])
                        if not do_avc:
                            nc.vector.tensor_copy(ycT_sb[i][:, q0:q0 + QC],
                                                  pct[:, 0:QC])
                    if do_avc:
                        ycps = ps_y.tile([DA, 2 * QC], f32, tag="y")
                        for hb in range(2):
                            nc.tensor.matmul(
                                ycps[:, hb * QC:(hb + 1) * QC],
                                vc_sb[0][:, (2 * i + hb) * DA:(2 * i + hb + 1) * DA],
                                pct[:, hb * QC:(hb + 1) * QC],
                                start=True, stop=True)
                        if not do_cnorm:
                            nc.vector.tensor_copy(
                                ycT_sb[i][0:DA, q0:q0 + QC], ycps[:, 0:QC])
                    if do_cnorm:
                        branches.append((ycps, ycT_sb))
                    # normalize branches into yT/ycT
                    for ps, dst in branches:
                        rec = nrm.tile([1, 2 * QC], f32, tag="rec")
                        nc.vector.reciprocal(rec[:], ps[D:DA, :])
                        bc = nrm.tile([D, 2 * QC], f32, tag="bc")
                        nc.sync.dma_start(
                            bc[:],
                            rec[0:1, :].unsqueeze(1).to_broadcast((1, D, 2 * QC)))
                        for hb in range(2):
                            nc.vector.tensor_tensor(
                                dst[i][hb * D:(hb + 1) * D, q0:q0 + QC],
                                ps[0:D, hb * QC:(hb + 1) * QC],
                                bc[:, hb * QC:(hb + 1) * QC], ALU.mult)

        # ---- gates, combine, output projection ----
        def gates_out():
            for o in range(PAIRS):
                for wname, src, dst, bn in (("g1", yT_sb, g1_sb, "g1"),
                                            ("g2", ycT_sb, g2_sb, "g2")):
                    ps = ps_b.tile([P, 512], f32, tag="psb")
                    for i in range(PAIRS):
                        nc.tensor.matmul(ps[:], w_sb[wname][:, i, P * o:P * o + P],
                                         src[i][:], start=(i == 0),
                                         stop=(i == PAIRS - 1))
                    bias = bv_sb[bn][:, o:o + 1] if has_b[bn] else 0.0
                    nc.scalar.activation(dst[o][:], ps[:], AF.Sigmoid, bias=bias)
                t1 = work.tile([P, TQ], f16, tag="zt")
                nc.vector.tensor_tensor(t1[:], g1_sb[o][:], ycT_sb[o][:], ALU.mult)
                nc.vector.tensor_tensor(z_sb[o][:], g2_sb[o][:], yT_sb[o][:], ALU.mult)
                nc.vector.tensor_tensor(z_sb[o][:], z_sb[o][:], t1[:], ALU.add)
            for m in range(PAIRS):
                ps = ps_b.tile([P, 512], f32, tag="psb")
                for o in range(PAIRS):
                    nc.tensor.matmul(ps[:], z_sb[o][:, P * m:P * m + P],
                                     w_sb["p"][:, o, :], start=(o == 0),
                                     stop=(o == PAIRS - 1))
                osb = work.tile([P, C], f32, tag="osb")
                if has_b["p"]:
                    nc.vector.tensor_tensor(osb[:], ps[:], bv_sb["p"][:], ALU.add)
                else:
                    nc.vector.tensor_copy(osb[:], ps[:])
                nc.sync.dma_start(out_d[P * m:P * m + P, :], osb[:])

        for rep in range(max(1, repeat)):
            projections()
            if stage == 1:
                dump([qT_sb[0][:, 0:C], kT_sb[0][:, 0:C],
                      v_sb[0][:, 0:C], vc_sb[0][:, 0:C]])
                return
            if stage == 15:
                dump([t[:] for t in kcT_sb])
                return
            attention()
            if stage in (2, 31, 32):
                dump([t[:, 0:C] for t in yT_sb])
                return
            if stage == 3:
                dump([t[:, 0:C] for t in ycT_sb])
                return
            gates_out()

    with tile.TileContext(nc) as tc, ExitStack() as ctx:
        emit(tc, ctx)
    nc.compile()
    _cache[key] = nc
    return nc


def prepare(inputs, stage=4, repeat=1):
    """Host-side prep: analyze mask, build program + per-core input maps."""
    x = np.asarray(inputs["x"], np.float32)
    c = np.asarray(inputs["c"], np.float32)
    attn_mask = np.asarray(inputs["attn_mask"])
    padding_mask = np.asarray(inputs["padding_mask"])
    W = {n: np.asarray(inputs["W" + n], np.float32)
         for n in ["q", "k", "v", "kc", "vc", "g1", "g2", "p"]}
    bvec = {n: np.asarray(inputs["b" + n], np.float32)
            for n in ["q", "k", "v", "kc", "vc", "g1", "g2", "p"]}

    scale = 1.0 / np.sqrt(D)
    W = dict(W)
    W["q"] = W["q"] * scale          # fold attention scale into Wq
    bq = bvec["q"] * scale

    mask2 = np.asarray(attn_mask).reshape(T, T)  # [q, k]
    ext_chunk = []
    for qc in range(NCHUNK):
        vis = mask2[qc * QC:(qc + 1) * QC, :].any(axis=0)
        last = int(np.nonzero(vis)[0].max()) if vis.any() else 0
        ext_chunk.append(last // KT + 1)
    ext = []
    for pos in range(2):
        e = max(ext_chunk[_chunks_of_core(j)[pos]] for j in range(4))
        ext.append(-(-e // GROUP) * GROUP)
    def _slot_needs(pos, s):
        for j in range(4):
            qc = _chunks_of_core(j)[pos]
            if s >= ext_chunk[qc]:
                return True
            blk = mask2[qc * QC:(qc + 1) * QC, s * KT:(s + 1) * KT]
            if not blk.all():
                return True
        return False

    # mask "units" cover one GROUP of k-slots (0/1 multiply on exp output)
    bias_slots = []
    for pos in range(2):
        for g in range(ext[pos] // GROUP):
            if any(_slot_needs(pos, g * GROUP + s4) for s4 in range(GROUP)):
                bias_slots.append((pos, g))

    has_b = {n: bool(np.any(bvec[n] != 0)) for n in bvec}
    nc = build_program(ext, bias_slots, has_b, stage=stage, repeat=repeat)

    w16 = {n: W[n].astype(np.float16) for n in W}
    ident = np.eye(P, dtype=np.float16)
    in_maps = []
    for core in range(8):
        b, j = divmod(core, 4)
        ca, cb = _chunks_of_core(j)
        xT = np.ascontiguousarray(x[b].T).astype(np.float16)        # [C, T]
        cols = np.r_[ca * QC:(ca + 1) * QC, cb * QC:(cb + 1) * QC]
        xqT = np.ascontiguousarray(xT[:, cols])
        cT = np.zeros((C, MP), np.float16)
        cT[:, :M] = c[b].T
        pad = np.zeros((P, 1), np.float32)
        pad[:M, 0] = np.where(padding_mask[b] != 0, 0.0, NEG)
        im = {"xT": xT, "xqT": xqT, "cT": cT, "ident": ident, "padb": pad}
        for n in w16:
            im["w" + n] = w16[n]
        if bias_slots:
            bm = np.empty((len(bias_slots), P, GROUP * QC), np.float16)
            for n, (pos, g) in enumerate(bias_slots):
                qc = (ca, cb)[pos]
                for e in range(GROUP):
                    s = g * GROUP + e
                    blk = mask2[qc * QC:(qc + 1) * QC, s * KT:(s + 1) * KT]
                    bm[n, :, e * QC:(e + 1) * QC] = np.where(
                        blk.T, 1.0, 0.0).astype(np.float16)
            im["biasm"] = bm
        for n in ["q", "k", "kc", "g1", "g2"]:
            if has_b[n]:
                v = (bq if n == "q" else bvec[n])
                im["b" + n] = np.ascontiguousarray(
                    v.reshape(KI, P).T).astype(np.float32)
        for n in ["v", "vc", "p"]:
            if has_b[n]:
                im["b" + n] = bvec[n].reshape(1, C).astype(np.float16)
        in_maps.append(im)
    return nc, in_maps


def kernel(**inputs):
    nc, in_maps = prepare(inputs)
    res = bass_utils.run_bass_kernel_spmd(nc, in_maps, core_ids=list(range(8)))
    out = np.empty((B, T, C), np.float32)
    for core in range(8):
        b, j = divmod(core, 4)
        ca, cb = _chunks_of_core(j)
        o = res.results[core]["out"]
        out[b, ca * QC:(ca + 1) * QC] = o[:QC]
        out[b, cb * QC:(cb + 1) * QC] = o[QC:]
    return out



# revision 13
# speedup vs baseline: 1.1887x; 1.1887x over previous
"""ConditionGateAttention Trainium2 kernel (v2).

Gated dual-attention block: causal self-attention + cross-attention to a
77-token condition, sigmoid cross-gating, output projection.

  B=2, T=2048, M=77, C=512, H=8 heads, D=64.

Sharding (8 cores): core = (b=core//4, j=core%4). Queries of batch b are
sorted by causal extent (host-side) and dealt round-robin to the 4 cores
in 8 "positions" of 64 queries each; position c needs keys only up to a
uniform extent E[c] (2(c+1) k-tiles for the causal mask), so every core
does the exact balanced share of causal work (144 k-tile units vs 192 for
contiguous-chunk sharding) with a program-uniform shape. K/V are computed
for the full batch locally (no collectives).

Precision: q/k/kc projections run in fp8e4(e4m3) DoubleRow mode (weights
scaled x16 on host to dodge fp8 subnormals; rescaled during the PSUM
eviction) - 4x fewer PE cycles than fp32, 2x fewer than fp16. Attention
(QK/AV), v, gates and output projection stay fp16 (validated: rel err
~5.6e-3 vs fp32 reference; fp8 probabilities/v push past the 2e-2 gate).

Softmax: logits for 16 (pos, ktile) slots share one [128, 16*64] PSUM
group -> one Exp per group on ACT. Fully-visible slots are exp'd as-is;
all partially-masked slots are packed into the trailing group(s) and
multiplied by a per-core 0/1 mask tile (one DVE multiply per mask group).
Denominators ride along AV as a ones-column on V; normalization is
reciprocal (DVE) -> ones-matmul partition-broadcast (PE, fp32r) -> fused
multiply+downcast (DVE). V/VC PSUM evictions run on the otherwise-idle
GPSIMD engine to keep DVE off the critical path.
"""
import numpy as np
import ml_dtypes
from contextlib import ExitStack

import concourse.bass as bass
import concourse.tile as tile
from concourse import bacc, mybir
from concourse import bass_utils

B, T, M, C, H = 2, 2048, 77, 512, 8
D = C // H            # 64
P = 128
KI = C // P           # 4 fp16 contraction chunks
KP = C // 256         # 2 fp8 DoubleRow contraction chunks (256 each)
PAIRS = H // 2        # pair i = heads 2i,2i+1 = C rows 128i..128i+128
NPOS = 8              # query positions per core
QP = 64               # queries per position
TQ = NPOS * QP        # 512 queries per core
KT = 128              # k-tile size
GS = 16               # slots per exp/psum group ([128, 1024] fp32 = 2 banks)
NEG = -30000.0
MP = 128              # condition length padded to 128
DA = D + 1            # V augmented with ones-column
WS = 16.0             # host-side fp8 weight scale

f8 = mybir.dt.float8e4
f16 = mybir.dt.float16
f32 = mybir.dt.float32
f32r = mybir.dt.float32r
AF = mybir.ActivationFunctionType
ALU = mybir.AluOpType
DR = mybir.MatmulPerfMode.DoubleRow

_cache = {}


def build_program(slots, group_specs, first_c, last_c, kv_tiles, has_b):
    """slots: tuple of (pos, ktile) in emission order (uniform across cores).
    group_specs: tuple of (offset, size, mask_col | None).
    first_c/last_c: slot index of first/last slot of each pos (AV start/stop).
    kv_tiles: number of 128-token k/v tiles to project.
    """
    key = (slots, group_specs, first_c, last_c, kv_tiles,
           tuple(sorted(has_b.items())))
    if key in _cache:
        return _cache[key]

    KV = kv_tiles * KT
    npart = sum(g[1] for g in group_specs if g[2] is not None)

    nc = bacc.Bacc("TRN2", num_devices=8, debug=False)

    xq8_d = nc.dram_tensor("xq8", [P, KP, 2, TQ], f8, kind="ExternalInput").ap()
    x8_d = nc.dram_tensor("x8", [P, KP, 2, KV], f8, kind="ExternalInput").ap()
    xT_d = nc.dram_tensor("xT", [P, KI, KV], f16, kind="ExternalInput").ap()
    c8_d = nc.dram_tensor("c8", [P, KP, 2, MP], f8, kind="ExternalInput").ap()
    cT_d = nc.dram_tensor("cT", [P, KI, MP], f16, kind="ExternalInput").ap()
    w8_d = {n: nc.dram_tensor(f"w8{n}", [P, KP, 2, C], f8, kind="ExternalInput").ap()
            for n in ["q", "k", "kc"]}
    w16_d = {n: nc.dram_tensor(f"w{n}", [P, KI, C], f16, kind="ExternalInput").ap()
             for n in ["v", "vc", "g1", "g2", "p"]}
    if npart:
        mask_d = nc.dram_tensor("maskm", [P, npart * QP], f16, kind="ExternalInput").ap()
    pad_d = nc.dram_tensor("padb", [P, 1], f32, kind="ExternalInput").ap()
    bv_d = {}
    for n in ["q", "k", "kc", "g1", "g2"]:
        if has_b[n]:
            bv_d[n] = nc.dram_tensor(f"b{n}", [P, PAIRS], f32, kind="ExternalInput").ap()
    for n in ["v", "vc", "p"]:
        if has_b[n]:
            bv_d[n] = nc.dram_tensor(f"b{n}", [1, C], f16, kind="ExternalInput").ap()
    out_d = nc.dram_tensor("out", [TQ, C], f16, kind="ExternalOutput").ap()

    def emit(tc, ctx):
        consts = ctx.enter_context(tc.tile_pool(name="consts", bufs=1))
        acts = ctx.enter_context(tc.tile_pool(name="acts", bufs=1))
        work = ctx.enter_context(tc.tile_pool(name="work", bufs=4))
        nrm = ctx.enter_context(tc.tile_pool(name="nrm", bufs=4))
        ps_a = ctx.enter_context(tc.tile_pool(name="ps_a", bufs=2, space="PSUM"))
        ps_b = ctx.enter_context(tc.tile_pool(name="ps_b", bufs=2, space="PSUM"))
        ps_y = ctx.enter_context(tc.tile_pool(name="ps_y", bufs=2, space="PSUM"))

        # ---- input loads, in consumption order ----
        xq8 = consts.tile([P, KP, 2, TQ], f8, name="xq8")
        nc.sync.dma_start(xq8[:], xq8_d)
        w8 = {n: consts.tile([P, KP, 2, C], f8, name=f"w8{n}") for n in w8_d}
        nc.sync.dma_start(w8["q"][:], w8_d["q"])
        c8 = consts.tile([P, KP, 2, MP], f8, name="c8")
        nc.sync.dma_start(c8[:], c8_d)
        nc.sync.dma_start(w8["kc"][:], w8_d["kc"])
        x8 = consts.tile([P, KP, 2, KV], f8, name="x8")
        nc.sync.dma_start(x8[:], x8_d)
        nc.sync.dma_start(w8["k"][:], w8_d["k"])
        xT = consts.tile([P, KI, KV], f16, name="xT")
        nc.sync.dma_start(xT[:], xT_d)
        w16 = {n: consts.tile([P, KI, C], f16, name=f"w{n}") for n in w16_d}
        nc.sync.dma_start(w16["v"][:], w16_d["v"])
        cT = consts.tile([P, KI, MP], f16, name="cT")
        nc.sync.dma_start(cT[:], cT_d)
        nc.sync.dma_start(w16["vc"][:], w16_d["vc"])
        if npart:
            maskm = consts.tile([P, npart * QP], f16, name="maskm")
            nc.sync.dma_start(maskm[:], mask_d)
        pad_sb = consts.tile([P, 1], f32, name="padb")
        nc.sync.dma_start(pad_sb[:], pad_d)
        zcol = consts.tile([P, DA], f16, name="zcol")
        nc.vector.memset(zcol[:], 0.0)
        for n in ["g1", "g2", "p"]:
            nc.sync.dma_start(w16[n][:], w16_d[n])
        bv = {}
        for n, d in bv_d.items():
            if n in ("v", "vc", "p"):
                bv[n] = consts.tile([P, C], f16, name=f"b{n}")
                nc.sync.dma_start(bv[n][:],
                                  d[0:1, :].unsqueeze(1).to_broadcast((1, P, C)))
            else:
                bv[n] = consts.tile([P, PAIRS], f32, name=f"b{n}")
                nc.sync.dma_start(bv[n][:], d)

        # ---- persistent activation tiles ----
        q16 = acts.tile([P, PAIRS, TQ], f16, name="q16")
        k16 = acts.tile([P, PAIRS, KV], f16, name="k16")
        kc16 = acts.tile([P, PAIRS, MP], f16, name="kc16")
        v_sb = [acts.tile([P, H * DA], f16, name=f"v{m}") for m in range(kv_tiles)]
        vc_sb = acts.tile([P, H * DA], f16, name="vc")
        y16 = acts.tile([P, PAIRS, TQ], f16, name="y16")
        yc16 = acts.tile([P, PAIRS, TQ], f16, name="yc16")
        g1t = acts.tile([P, PAIRS, TQ], f16, name="g1t")
        g2t = acts.tile([P, PAIRS, TQ], f16, name="g2t")
        zt = acts.tile([P, PAIRS, TQ], f16, name="zt")

        # ---- fp8 DoubleRow projection: out[cols 128i..](tok) ----
        def proj8(wname, rhs8, n_free, dst, i, invs):
            # dst[:, i, :] <- (W8.T @ rhs8) * invs ; tiles of <=512 psum cols
            for t0 in range(0, n_free, 512):
                fw = min(512, n_free - t0)
                ps = ps_b.tile([P, 512], f32, tag="psb")
                for tt in range(t0, t0 + fw, 256):
                    cw = min(256, t0 + fw - tt)
                    for kp in range(KP):
                        nc.tensor.matmul(
                            ps[:, tt - t0:tt - t0 + cw],
                            w8[wname][:, kp, :, P * i:P * i + P],
                            rhs8[:, kp, :, tt:tt + cw],
                            start=(kp == 0), stop=(kp == KP - 1),
                            perf_mode=DR)
                if has_b[wname]:
                    nc.scalar.activation(dst[:, i, t0:t0 + fw], ps[:, 0:fw],
                                         AF.Identity, bias=bv[wname][:, i:i + 1],
                                         scale=invs)
                else:
                    nc.vector.tensor_scalar_mul(dst[:, i, t0:t0 + fw],
                                                ps[:, 0:fw], invs)

        # ---- fp16 V projection (natural layout, ones-augmented) ----
        def vproj(wname, src, rows, m, out_tile, ones_rows):
            ps = ps_b.tile([P, 512], f32, tag="psb")
            for ki in range(KI):
                nc.tensor.matmul(ps[:], src[:, ki, m * P:m * P + P],
                                 w16[wname][:, ki, :],
                                 start=(ki == 0), stop=(ki == KI - 1))
            dst = out_tile.rearrange("p (h e) -> p h e", e=DA)
            nc.vector.tensor_copy(dst[:, :, 0:D],
                                  ps[:].rearrange("p (h e) -> p h e", e=D))
            if has_b[wname]:
                nc.vector.tensor_tensor(
                    dst[:, :, 0:D], dst[:, :, 0:D],
                    bv[wname][:].rearrange("p (h e) -> p h e", e=D), ALU.add)
            if ones_rows < P:
                nc.gpsimd.memset(dst[:, :, D:DA], 0.0)
            nc.gpsimd.memset(dst[0:ones_rows, :, D:DA], 1.0)

        # scale comp: w8 carry x16; q additionally carries 1/sqrt(D)
        invq = 1.0 / (WS * float(np.sqrt(D)))
        invk = 1.0 / WS

        def kproj(i):
            proj8("k", x8, KV, k16, i, invk)

        # ---- attention for (pair i, head h) ----
        qscale = 1.0  # scale already folded into q

        def attention(i, h, extra=None):
            b0 = h * D
            hcol = (2 * i + h) * DA
            # AV windows for the 8 positions interleave inside one psum
            # zero-region, which start-bit zeroing can't express: zero the
            # tile with one cheap PE matmul (zeros stationary) and accumulate
            # with start=False throughout.
            yps = ps_y.tile([DA, TQ], f32, tag="y")
            nc.tensor.matmul(yps[:], zcol[:], q16[:, i, :],
                             start=True, stop=False, skip_group_check=True)

            def qk_group(g):
                off, size, mcol = group_specs[g]
                lg = ps_a.tile([P, size * QP], f32, tag="lg")
                for n in range(size):
                    cc, s = slots[off + n]
                    nc.tensor.matmul(
                        lg[:, n * QP:(n + 1) * QP],
                        k16[b0:b0 + D, i, s * KT:(s + 1) * KT],
                        q16[b0:b0 + D, i, cc * QP:(cc + 1) * QP],
                        start=True, stop=True)
                pt = work.tile([P, size * QP], f16, tag="pt")
                nc.scalar.activation(pt[:], lg[:], AF.Exp)
                if mcol is not None:
                    nc.vector.tensor_tensor(
                        pt[:], pt[:], maskm[:, mcol:mcol + size * QP], ALU.mult)
                return pt

            def av_group(g, pt):
                off, size, _ = group_specs[g]
                for n in range(size):
                    cc, s = slots[off + n]
                    nc.tensor.matmul(
                        yps[:, cc * QP:(cc + 1) * QP],
                        v_sb[s][:, hcol:hcol + DA],
                        pt[:, n * QP:(n + 1) * QP],
                        start=False, stop=(off + n == last_c[cc]),
                        skip_group_check=True)

            NG = len(group_specs)
            pts = {}
            for g in range(NG):
                pts[g] = qk_group(g)
                if g == 0 and extra is not None:
                    extra()  # interleave next pair's k-projection
                if g >= 2:
                    av_group(g - 2, pts.pop(g - 2))
            for g in range(max(0, NG - 2), NG):
                av_group(g, pts.pop(g))

            # cross-attention for this head
            scp = ps_b.tile([P, TQ], f32, tag="psb")
            nc.tensor.matmul(scp[:], kc16[b0:b0 + D, i, :],
                             q16[b0:b0 + D, i, :], start=True, stop=True)
            pct = work.tile([P, TQ], f16, tag="pt")
            nc.scalar.activation(pct[:], scp[:], AF.Exp, bias=pad_sb[:, 0:1])
            ycps = ps_y.tile([DA, TQ], f32, tag="y")
            nc.tensor.matmul(ycps[:], vc_sb[:, hcol:hcol + DA], pct[:],
                             start=True, stop=True)

            # normalize both branches: recip -> DMA broadcast -> fused mult
            for ps, dst in ((yps, y16), (ycps, yc16)):
                rec = nrm.tile([1, TQ], f32, tag="rec")
                nc.vector.reciprocal(rec[:], ps[D:DA, :])
                bc = nrm.tile([D, TQ], f32, tag="bc")
                nc.sync.dma_start(
                    bc[:], rec[0:1, :].unsqueeze(1).to_broadcast((1, D, TQ)))
                nc.vector.tensor_tensor(dst[b0:b0 + D, i, :], ps[0:D, :],
                                        bc[:], ALU.mult)

        # ---- gates, combine, output projection ----
        def gates_out():
            for o in range(PAIRS):
                for wname, src, dstt, bn in (("g1", y16, g1t, "g1"),
                                             ("g2", yc16, g2t, "g2")):
                    ps = ps_b.tile([P, TQ], f32, tag="psb")
                    for i in range(PAIRS):
                        nc.tensor.matmul(ps[:], w16[wname][:, i, P * o:P * o + P],
                                         src[:, i, :], start=(i == 0),
                                         stop=(i == PAIRS - 1))
                    bias = bv[bn][:, o:o + 1] if has_b[bn] else 0.0
                    nc.scalar.activation(dstt[:, o, :], ps[:], AF.Sigmoid, bias=bias)
                t1 = work.tile([P, TQ], f16, tag="zt")
                nc.vector.tensor_tensor(t1[:], g1t[:, o, :], yc16[:, o, :], ALU.mult)
                nc.vector.tensor_tensor(zt[:, o, :], g2t[:, o, :], y16[:, o, :],
                                        ALU.mult)
                nc.vector.tensor_tensor(zt[:, o, :], zt[:, o, :], t1[:], ALU.add)
            for m in range(PAIRS):
                ps = ps_b.tile([P, C], f32, tag="psb")
                for o in range(PAIRS):
                    nc.tensor.matmul(ps[:], zt[:, o, P * m:P * m + P],
                                     w16["p"][:, o, :], start=(o == 0),
                                     stop=(o == PAIRS - 1))
                osb = work.tile([P, C], f16, tag="osb")
                if has_b["p"]:
                    nc.vector.tensor_tensor(osb[:], ps[:], bv["p"][:], ALU.add)
                else:
                    nc.vector.tensor_copy(osb[:], ps[:])
                nc.sync.dma_start(out_d[P * m:P * m + P, :], osb[:])

        # ---- schedule ----
        for i in range(PAIRS):
            proj8("q", xq8, TQ, q16, i, invq)
        for i in range(PAIRS):
            proj8("kc", c8, MP, kc16, i, invk)
        kproj(0)
        vproj("vc", cT, MP, 0, vc_sb, M)
        for m in range(kv_tiles):
            vproj("v", xT, KV, m, v_sb[m], P)
        for i in range(PAIRS):
            extra = (lambda i=i: kproj(i + 1)) if i + 1 < PAIRS else None
            attention(i, 0, extra=extra)
            attention(i, 1)
        gates_out()

    with tile.TileContext(nc) as tc, ExitStack() as ctx:
        with nc.allow_low_precision("fp32r normalize broadcast"):
            emit(tc, ctx)
    nc.compile()
    _cache[key] = nc
    return nc


def _plan(mask2):
    """Derive the uniform attention schedule from the (shared) attn mask."""
    vis = mask2 != 0
    need = np.where(vis.any(1), vis.shape[1] - np.argmax(vis[:, ::-1], 1), 1)
    order = np.argsort(need, kind="stable")
    E, fulls, parts = [], [], []
    for c in range(NPOS):
        qc = order[4 * QP * c:4 * QP * (c + 1)]
        e = -(-int(need[qc].max()) // KT)
        E.append(e)
        f, p = [], []
        for s in range(e):
            blk = vis[qc][:, s * KT:(s + 1) * KT]
            (f if blk.all() else p).append(s)
        fulls.append(f)
        parts.append(p)
    fslots = [(c, s) for c in range(NPOS) for s in fulls[c]]
    pslots = [(c, s) for c in range(NPOS) for s in parts[c]]
    slots = fslots + pslots
    group_specs = []
    for o in range(0, len(fslots), GS):
        group_specs.append((o, min(GS, len(fslots) - o), None))
    for o in range(0, len(pslots), GS):
        group_specs.append((len(fslots) + o, min(GS, len(pslots) - o), o * QP))
    first_c = [min(n for n, (c, _) in enumerate(slots) if c == cc)
               for cc in range(NPOS)]
    last_c = [max(n for n, (c, _) in enumerate(slots) if c == cc)
              for cc in range(NPOS)]
    kv_tiles = max(E)
    return order, slots, group_specs, first_c, last_c, kv_tiles


def core_rows(core, order=None):
    """Global query indices handled by `core`, in output-row order."""
    if order is None:
        order = _plan(np.tril(np.ones((T, T), np.int64)))[0]
    j = core % 4
    return np.concatenate([order[4 * QP * c + j:4 * QP * (c + 1):4]
                           for c in range(NPOS)])


def _dr8(a, scale=1.0):
    # [C, N] f32 -> [128, KP, 2, N] fp8 DoubleRow layout
    Cr, N = a.shape
    return np.ascontiguousarray(
        (a * scale).reshape(KP, 2, P, N).transpose(2, 0, 1, 3)
    ).astype(ml_dtypes.float8_e4m3)


def _chunk16(a):
    # [C, N] f32 -> [128, KI, N] fp16
    Cr, N = a.shape
    return np.ascontiguousarray(
        a.reshape(KI, P, N).transpose(1, 0, 2)).astype(np.float16)


def prepare(inputs):
    x = np.asarray(inputs["x"], np.float32)
    c = np.asarray(inputs["c"], np.float32)
    attn_mask = np.asarray(inputs["attn_mask"])
    padding_mask = np.asarray(inputs["padding_mask"])
    W = {n: np.asarray(inputs["W" + n], np.float32)
         for n in ["q", "k", "v", "kc", "vc", "g1", "g2", "p"]}
    bvec = {n: np.asarray(inputs["b" + n], np.float32)
            for n in ["q", "k", "v", "kc", "vc", "g1", "g2", "p"]}
    has_b = {n: bool(np.any(bvec[n] != 0)) for n in bvec}

    mask2 = np.asarray(attn_mask).reshape(T, T)
    order, slots, group_specs, first_c, last_c, kv_tiles = _plan(mask2)
    npart = sum(g[1] for g in group_specs if g[2] is not None)

    nc = build_program(tuple(slots), tuple(group_specs), tuple(first_c),
                       tuple(last_c), kv_tiles, has_b)

    KV = kv_tiles * KT
    w8 = {n: _dr8(W[n], WS) for n in ["q", "k", "kc"]}
    w16 = {n: _chunk16(W[n]) for n in ["v", "vc", "g1", "g2", "p"]}
    pslots = slots[len(slots) - npart:]

    in_maps = []
    for core in range(8):
        b, j = divmod(core, 4)
        rows = core_rows(core, order)
        xT = x[b].T.astype(np.float32)            # [C, T]
        xq = np.ascontiguousarray(xT[:, rows])
        cTf = np.zeros((C, MP), np.float32)
        cTf[:, :M] = c[b].T
        pad = np.zeros((P, 1), np.float32)
        pad[:M, 0] = np.where(padding_mask[b] != 0, 0.0, NEG)
        im = {
            "xq8": _dr8(xq), "x8": _dr8(xT[:, :KV]),
            "xT": _chunk16(xT[:, :KV]),
            "c8": _dr8(cTf), "cT": _chunk16(cTf),
            "padb": pad,
        }
        for n in w8:
            im["w8" + n] = w8[n]
        for n in w16:
            im["w" + n] = w16[n]
        if npart:
            mm = np.zeros((P, npart * QP), np.float16)
            for nslot, (cc, s) in enumerate(pslots):
                qrows = rows[cc * QP:(cc + 1) * QP]
                blk = mask2[qrows][:, s * KT:(s + 1) * KT]  # [QP, KT]
                mm[:, nslot * QP:(nslot + 1) * QP] = np.where(blk.T, 1.0, 0.0)
            im["maskm"] = mm
        for n in ["q", "k", "kc", "g1", "g2"]:
            if has_b[n]:
                v = bvec[n] * (1.0 / np.sqrt(D) if n == "q" else 1.0)
                im["b" + n] = np.ascontiguousarray(
                    v.reshape(PAIRS, P).T).astype(np.float32)
        for n in ["v", "vc", "p"]:
            if has_b[n]:
                im["b" + n] = bvec[n].reshape(1, C).astype(np.float16)
        in_maps.append(im)
    return nc, in_maps


def kernel(**inputs):
    nc, in_maps = prepare(inputs)
    res = bass_utils.run_bass_kernel_spmd(nc, in_maps, core_ids=list(range(8)))
    mask2 = np.asarray(inputs["attn_mask"]).reshape(T, T)
    order = _plan(mask2)[0]
    out = np.empty((B, T, C), np.float32)
    for core in range(8):
        b = core // 4
        rows = core_rows(core, order)
        out[b, rows] = res.results[core]["out"].astype(np.float32)
    return out


# revision 14
# speedup vs baseline: 1.2023x; 1.0115x over previous
"""ConditionGateAttention Trainium2 kernel (v3).

Gated dual-attention block: causal self-attention + cross-attention to a
77-token condition, sigmoid cross-gating, output projection.

  B=2, T=2048, M=77, C=512, H=8 heads, D=64.

Sharding (8 cores): core = (b=core//4, j=core%4). Queries of batch b are
sorted by causal extent (host-side) and dealt round-robin to the 4 cores
in 8 "positions" of 64 queries each; position c needs keys only up to a
uniform extent (2(c+1) k-tiles for the causal mask), so every core does
the exact balanced share of causal work (144 k-tile units vs 192 for
contiguous-chunk sharding) with a program-uniform shape. K/V are computed
for the full batch locally (no collectives).

Precision: q/k/kc projections run in fp8e4(e4m3) DoubleRow mode (weights
scaled x16 on host to dodge fp8 subnormals; rescaled during the PSUM
eviction). Attention (QK/AV), v, gates and output projection stay fp16
(validated ~5.6e-3 rel err; fp8 probabilities/v would blow the 2e-2 gate).

Schedule: per (pair, head): cross-attention QK+exp leads (fills ACT while
PE zeroes/fills the self path), then 16-slot exp groups with a lag-2
QK->exp->AV software pipeline. Partially-masked slots are packed into the
leading group(s) (trailing for pair0-h0 so its AV can start before v-proj
of late k-tiles lands) and masked with one GPSIMD multiply. Denominators
ride AV as a ones-column on V; self-branch PSUM is evicted to SBUF
immediately (frees the PSUM buffer for the next head) and normalized via
reciprocal -> DMA partition-broadcast -> fused multiply on DVE. V/VC
PSUM evictions run on ACT (idle during the projection lead-in); q/k/kc
rescale-evictions on DVE; per-pair y tiles let the gate matmuls start
while the last pair is still normalizing.
"""
import numpy as np
import ml_dtypes
from contextlib import ExitStack

import concourse.bass as bass
import concourse.tile as tile
from concourse import bacc, mybir
from concourse import bass_utils

B, T, M, C, H = 2, 2048, 77, 512, 8
D = C // H            # 64
P = 128
KI = C // P           # 4 fp16 contraction chunks
KP = C // 256         # 2 fp8 DoubleRow contraction chunks (256 each)
PAIRS = H // 2        # pair i = heads 2i,2i+1 = C rows 128i..128i+128
NPOS = 8              # query positions per core
QP = 64               # queries per position
TQ = NPOS * QP        # 512 queries per core
KT = 128              # k-tile size
GS = 16               # slots per exp/psum group ([128, 1024] fp32 = 2 banks)
NEG = -30000.0
MP = 128              # condition length padded to 128
DA = D + 1            # V augmented with ones-column
WS = 16.0             # host-side fp8 weight scale

f8 = mybir.dt.float8e4
f16 = mybir.dt.float16
f32 = mybir.dt.float32
AF = mybir.ActivationFunctionType
ALU = mybir.AluOpType
DR = mybir.MatmulPerfMode.DoubleRow

_cache = {}


def build_program(slots, group_specs, kv_tiles, has_b):
    """slots: tuple of (pos, ktile), partial-masked slots first (uniform
    across cores). group_specs: tuple of (offset, size, mask_col | None).
    kv_tiles: number of 128-token k/v tiles to project."""
    key = (slots, group_specs, kv_tiles, tuple(sorted(has_b.items())))
    if key in _cache:
        return _cache[key]

    KV = kv_tiles * KT
    npart = sum(g[1] for g in group_specs if g[2] is not None)
    NG = len(group_specs)

    nc = bacc.Bacc("TRN2", num_devices=8, debug=False)

    # fused fp8 input: [xq8 | w8q | c8 | w8kc] then [x8 | w8k]
    A0 = TQ + C + MP + C
    A1 = KV + C
    a8_d = nc.dram_tensor("a8", [P, KP, 2, A0], f8, kind="ExternalInput").ap()
    b8_d = nc.dram_tensor("b8", [P, KP, 2, A1], f8, kind="ExternalInput").ap()
    # fused fp16 input: [xT | wv] and [cT | wvc]
    xv_d = nc.dram_tensor("xv", [P, KI, KV + C], f16, kind="ExternalInput").ap()
    cv_d = nc.dram_tensor("cv", [P, KI, MP + C], f16, kind="ExternalInput").ap()
    g12p_d = nc.dram_tensor("g12p", [P, KI, 3 * C], f16, kind="ExternalInput").ap()
    if npart:
        mask_d = nc.dram_tensor("maskm", [P, npart * QP], f16, kind="ExternalInput").ap()
    pad_d = nc.dram_tensor("padb", [P, 1], f32, kind="ExternalInput").ap()
    bv_d = {}
    for n in ["q", "k", "kc", "g1", "g2"]:
        if has_b[n]:
            bv_d[n] = nc.dram_tensor(f"b{n}", [P, PAIRS], f32, kind="ExternalInput").ap()
    for n in ["v", "vc", "p"]:
        if has_b[n]:
            bv_d[n] = nc.dram_tensor(f"b{n}", [1, C], f16, kind="ExternalInput").ap()
    out_d = nc.dram_tensor("out", [TQ, C], f16, kind="ExternalOutput").ap()

    def emit(tc, ctx):
        consts = ctx.enter_context(tc.tile_pool(name="consts", bufs=1))
        acts = ctx.enter_context(tc.tile_pool(name="acts", bufs=1))
        work = ctx.enter_context(tc.tile_pool(name="work", bufs=4))
        nrm = ctx.enter_context(tc.tile_pool(name="nrm", bufs=4))
        ps_a = ctx.enter_context(tc.tile_pool(name="ps_a", bufs=2, space="PSUM"))
        ps_b = ctx.enter_context(tc.tile_pool(name="ps_b", bufs=2, space="PSUM"))
        ps_y = ctx.enter_context(tc.tile_pool(name="ps_y", bufs=2, space="PSUM"))

        # ---- input loads, in consumption order ----
        a8 = consts.tile([P, KP, 2, A0], f8, name="a8")
        nc.sync.dma_start(a8[:], a8_d)
        xq8 = a8[:, :, :, 0:TQ]
        w8q = a8[:, :, :, TQ:TQ + C]
        c8 = a8[:, :, :, TQ + C:TQ + C + MP]
        w8kc = a8[:, :, :, TQ + C + MP:]
        b8 = consts.tile([P, KP, 2, A1], f8, name="b8")
        half = (A1 // 2) // 64 * 64
        nc.sync.dma_start(b8[:, :, :, 0:half], b8_d[:, :, :, 0:half])
        nc.sync.dma_start(b8[:, :, :, half:], b8_d[:, :, :, half:])
        x8 = b8[:, :, :, 0:KV]
        w8k = b8[:, :, :, KV:]
        xv = consts.tile([P, KI, KV + C], f16, name="xv")
        xvh = (KV + C) // 2
        nc.sync.dma_start(xv[:, :, 0:xvh], xv_d[:, :, 0:xvh])
        nc.sync.dma_start(xv[:, :, xvh:], xv_d[:, :, xvh:])
        xT = xv[:, :, 0:KV]
        wv = xv[:, :, KV:]
        cv = consts.tile([P, KI, MP + C], f16, name="cv")
        nc.sync.dma_start(cv[:], cv_d)
        cT = cv[:, :, 0:MP]
        wvc = cv[:, :, MP:]
        if npart:
            maskm = consts.tile([P, npart * QP], f16, name="maskm")
            nc.sync.dma_start(maskm[:], mask_d)
        pad_sb = consts.tile([P, 1], f32, name="padb")
        nc.sync.dma_start(pad_sb[:], pad_d)
        g12p = consts.tile([P, KI, 3 * C], f16, name="g12p")
        nc.sync.dma_start(g12p[:], g12p_d)
        wg = {"g1": g12p[:, :, 0:C], "g2": g12p[:, :, C:2 * C],
              "p": g12p[:, :, 2 * C:]}
        zcol = consts.tile([P, DA], f16, name="zcol")
        nc.vector.memset(zcol[:], 0.0)
        bv = {}
        for n, d in bv_d.items():
            if n in ("v", "vc", "p"):
                bv[n] = consts.tile([P, C], f16, name=f"b{n}")
                nc.sync.dma_start(bv[n][:],
                                  d[0:1, :].unsqueeze(1).to_broadcast((1, P, C)))
            else:
                bv[n] = consts.tile([P, PAIRS], f32, name=f"b{n}")
                nc.sync.dma_start(bv[n][:], d)

        # ---- persistent activation tiles ----
        q16 = acts.tile([P, PAIRS, TQ], f16, name="q16")
        k16 = acts.tile([P, PAIRS, KV], f16, name="k16")
        kc16 = acts.tile([P, PAIRS, MP], f16, name="kc16")
        v_sb = [acts.tile([P, H * DA], f16, name=f"v{m}") for m in range(kv_tiles)]
        vc_sb = acts.tile([P, H * DA], f16, name="vc")
        y16 = [acts.tile([P, TQ], f16, name=f"y16_{i}") for i in range(PAIRS)]
        yc16 = [acts.tile([P, TQ], f16, name=f"yc16_{i}") for i in range(PAIRS)]
        g1t = [acts.tile([P, TQ], f16, name=f"g1_{o}") for o in range(PAIRS)]
        g2t = [acts.tile([P, TQ], f16, name=f"g2_{o}") for o in range(PAIRS)]
        zt = [acts.tile([P, TQ], f16, name=f"z_{o}") for o in range(PAIRS)]

        # ---- fp8 DoubleRow projection ----
        def proj8(w8t, rhs8, n_free, dst, i, invs, bname):
            for t0 in range(0, n_free, 512):
                fw = min(512, n_free - t0)
                ps = ps_b.tile([P, fw], f32, tag="psb", name="pj")
                for tt in range(t0, t0 + fw, 256):
                    cw = min(256, t0 + fw - tt)
                    for kp in range(KP):
                        nc.tensor.matmul(
                            ps[:, tt - t0:tt - t0 + cw],
                            w8t[:, kp, :, P * i:P * i + P],
                            rhs8[:, kp, :, tt:tt + cw],
                            start=(kp == 0), stop=(kp == KP - 1),
                            perf_mode=DR)
                if has_b[bname]:
                    nc.scalar.activation(dst[:, i, t0:t0 + fw], ps[:],
                                         AF.Identity, bias=bv[bname][:, i:i + 1],
                                         scale=invs)
                else:
                    nc.vector.tensor_scalar_mul(dst[:, i, t0:t0 + fw],
                                                ps[:], invs)

        # ---- fp16 V projection (natural layout, ones-augmented) ----
        def vproj(wt, src, m, out_tile, ones_rows, bname):
            ps = ps_b.tile([P, 512], f32, tag="psb", name="pv")
            for ki in range(KI):
                nc.tensor.matmul(ps[:], src[:, ki, m * P:m * P + P],
                                 wt[:, ki, :], start=(ki == 0),
                                 stop=(ki == KI - 1))
            dst = out_tile.rearrange("p (h e) -> p h e", e=DA)
            nc.scalar.copy(dst[:, :, 0:D],
                           ps[:].rearrange("p (h e) -> p h e", e=D))
            if has_b[bname]:
                nc.vector.tensor_tensor(
                    dst[:, :, 0:D], dst[:, :, 0:D],
                    bv[bname][:].rearrange("p (h e) -> p h e", e=D), ALU.add)
            if ones_rows < P:
                nc.gpsimd.memset(dst[:, :, D:DA], 0.0)
            nc.gpsimd.memset(dst[0:ones_rows, :, D:DA], 1.0)

        invq = 1.0 / (WS * float(np.sqrt(D)))
        invk = 1.0 / WS

        def kproj(i):
            proj8(w8k, x8, KV, k16, i, invk, "k")

        # ---- attention for (pair i, head h) ----
        def attention(i, h, extra=None, fulls_first=False):
            b0 = h * D
            hcol = (2 * i + h) * DA

            # cross-attention leads: fills ACT while the self path spins up
            scp = ps_b.tile([P, TQ], f32, tag="psb", name="scp")
            nc.tensor.matmul(scp[:], kc16[b0:b0 + D, i, :],
                             q16[b0:b0 + D, i, :], start=True, stop=True)
            pct = work.tile([P, TQ], f16, tag="pt", name="pct")
            nc.scalar.activation(pct[:], scp[:], AF.Exp, bias=pad_sb[:, 0:1])
            if extra is not None:
                extra()  # next pair's k-projection, off the critical path

            # self-attention: zero the accumulator (start-bit zeroing can't
            # express 8 interleaved per-position windows), then lag-2
            # QK -> exp -> AV pipeline over the slot groups.
            yps = ps_y.tile([DA, TQ], f32, tag="y", name="yps")
            nc.tensor.matmul(yps[:], zcol[:], q16[:, i, :],
                             start=True, stop=False, skip_group_check=True)

            order = list(range(NG))
            if fulls_first:
                order = [g for g in order if group_specs[g][2] is None] + \
                        [g for g in order if group_specs[g][2] is not None]
            lastmap = {}
            for g in order:
                off, size, _ = group_specs[g]
                for n in range(size):
                    lastmap[slots[off + n][0]] = off + n

            def qk_group(g):
                off, size, mcol = group_specs[g]
                lg = ps_a.tile([P, size * QP], f32, tag="lg", name="lg")
                for n in range(size):
                    cc, s = slots[off + n]
                    nc.tensor.matmul(
                        lg[:, n * QP:(n + 1) * QP],
                        k16[b0:b0 + D, i, s * KT:(s + 1) * KT],
                        q16[b0:b0 + D, i, cc * QP:(cc + 1) * QP],
                        start=True, stop=True)
                pt = work.tile([P, size * QP], f16, tag="pt", name="pt")
                nc.scalar.activation(pt[:], lg[:], AF.Exp)
                if mcol is not None:
                    nc.gpsimd.tensor_tensor(
                        pt[:], pt[:], maskm[:, mcol:mcol + size * QP], ALU.mult)
                return pt

            def av_group(g, pt):
                off, size, _ = group_specs[g]
                for n in range(size):
                    cc, s = slots[off + n]
                    nc.tensor.matmul(
                        yps[:, cc * QP:(cc + 1) * QP],
                        v_sb[s][:, hcol:hcol + DA],
                        pt[:, n * QP:(n + 1) * QP],
                        start=False, stop=(off + n == lastmap[cc]),
                        skip_group_check=True)

            pts = {}
            ycps = None
            for gi, g in enumerate(order):
                pts[g] = qk_group(g)
                if gi == 0:
                    # cross AV: pct is ready by now; separate PSUM tile
                    ycps = ps_y.tile([DA, TQ], f32, tag="y", name="ycps")
                    nc.tensor.matmul(ycps[:], vc_sb[:, hcol:hcol + DA],
                                     pct[:], start=True, stop=True)
                    # cross normalize (PSUM-direct; frees ycps mid-head)
                    norm_branch(ycps, yc16[i], b0, evict=False)
                if gi >= 2:
                    av_group(order[gi - 2], pts.pop(order[gi - 2]))
            for gi in range(max(0, NG - 2), NG):
                av_group(order[gi], pts.pop(order[gi]))

            # self normalize: evict PSUM immediately so the next head's
            # accumulator allocation doesn't wait on the normalize chain.
            norm_branch(yps, y16[i], b0, evict=True)

        def norm_branch(ps, dst, b0, evict):
            if evict:
                yraw = nrm.tile([DA, TQ], f32, tag="yraw")
                nc.vector.tensor_copy(yraw[:], ps[:])
                src = yraw
            else:
                src = ps
            rec = nrm.tile([1, TQ], f32, tag="rec")
            nc.vector.reciprocal(rec[:], src[D:DA, :])
            bc = nrm.tile([D, TQ], f32, tag="bc")
            nc.sync.dma_start(
                bc[:], rec[0:1, :].unsqueeze(1).to_broadcast((1, D, TQ)))
            nc.vector.tensor_tensor(dst[b0:b0 + D, :], src[0:D, :],
                                    bc[:], ALU.mult)

        # ---- gates, combine, output projection ----
        def gates_out():
            for o in range(PAIRS):
                for wname, srcs, dstt, bn in (("g1", y16, g1t, "g1"),
                                              ("g2", yc16, g2t, "g2")):
                    ps = ps_b.tile([P, TQ], f32, tag="psb", name="pg")
                    for i in range(PAIRS):
                        nc.tensor.matmul(ps[:], wg[wname][:, i, P * o:P * o + P],
                                         srcs[i][:], start=(i == 0),
                                         stop=(i == PAIRS - 1))
                    bias = bv[bn][:, o:o + 1] if has_b[bn] else 0.0
                    nc.scalar.activation(dstt[o][:], ps[:], AF.Sigmoid, bias=bias)
                t1 = work.tile([P, TQ], f16, tag="zt")
                nc.vector.tensor_tensor(t1[:], g1t[o][:], yc16[o][:], ALU.mult)
                nc.vector.tensor_tensor(zt[o][:], g2t[o][:], y16[o][:], ALU.mult)
                nc.vector.tensor_tensor(zt[o][:], zt[o][:], t1[:], ALU.add)
            for m in range(PAIRS):
                ps = ps_b.tile([P, C], f32, tag="psb", name="po")
                for o in range(PAIRS):
                    nc.tensor.matmul(ps[:], zt[o][:, P * m:P * m + P],
                                     wg["p"][:, o, :], start=(o == 0),
                                     stop=(o == PAIRS - 1))
                osb = work.tile([P, C], f16, tag="osb")
                if has_b["p"]:
                    nc.vector.tensor_tensor(osb[:], ps[:], bv["p"][:], ALU.add)
                elif m % 2 == 0:
                    nc.scalar.copy(osb[:], ps[:])
                else:
                    nc.vector.tensor_copy(osb[:], ps[:])
                nc.sync.dma_start(out_d[P * m:P * m + P, :], osb[:])

        # ---- schedule ----
        for i in range(PAIRS):
            proj8(w8q, xq8, TQ, q16, i, invq, "q")
        for i in range(PAIRS):
            proj8(w8kc, c8, MP, kc16, i, invk, "kc")
        kproj(0)
        vproj(wvc, cT, 0, vc_sb, M, "vc")
        for m in range(kv_tiles):
            vproj(wv, xT, m, v_sb[m], P, "v")
        for i in range(PAIRS):
            extra = (lambda i=i: kproj(i + 1)) if i + 1 < PAIRS else None
            attention(i, 0, extra=extra, fulls_first=(i == 0))
            attention(i, 1)
        gates_out()

    with tile.TileContext(nc) as tc, ExitStack() as ctx:
        emit(tc, ctx)
    nc.compile()
    _cache[key] = nc
    return nc


def _plan(mask2):
    """Derive the uniform attention schedule from the (shared) attn mask."""
    vis = mask2 != 0
    need = np.where(vis.any(1), vis.shape[1] - np.argmax(vis[:, ::-1], 1), 1)
    order = np.argsort(need, kind="stable")
    E, fulls, parts = [], [], []
    for c in range(NPOS):
        qc = order[4 * QP * c:4 * QP * (c + 1)]
        e = -(-int(need[qc].max()) // KT)
        E.append(e)
        f, p = [], []
        for s in range(e):
            blk = vis[qc][:, s * KT:(s + 1) * KT]
            (f if blk.all() else p).append(s)
        fulls.append(f)
        parts.append(p)
    pslots = [(c, s) for c in range(NPOS) for s in parts[c]]
    fslots = [(c, s) for c in range(NPOS) for s in fulls[c]]
    slots = pslots + fslots
    group_specs = []
    for o in range(0, len(pslots), GS):
        group_specs.append((o, min(GS, len(pslots) - o), o * QP))
    for o in range(0, len(fslots), GS):
        group_specs.append((len(pslots) + o, min(GS, len(fslots) - o), None))
    kv_tiles = max(E)
    return order, slots, group_specs, kv_tiles


def core_rows(core, order=None):
    """Global query indices handled by `core`, in output-row order."""
    if order is None:
        order = _plan(np.tril(np.ones((T, T), np.int64)))[0]
    j = core % 4
    return np.concatenate([order[4 * QP * c + j:4 * QP * (c + 1):4]
                           for c in range(NPOS)])


def _dr8(a, scale=1.0):
    # [C, N] f32 -> [128, KP, 2, N] fp8 DoubleRow layout
    Cr, N = a.shape
    return np.ascontiguousarray(
        (a * scale).reshape(KP, 2, P, N).transpose(2, 0, 1, 3)
    ).astype(ml_dtypes.float8_e4m3)


def _chunk16(a):
    # [C, N] f32 -> [128, KI, N] fp16
    Cr, N = a.shape
    return np.ascontiguousarray(
        a.reshape(KI, P, N).transpose(1, 0, 2)).astype(np.float16)


def prepare(inputs):
    x = np.asarray(inputs["x"], np.float32)
    c = np.asarray(inputs["c"], np.float32)
    attn_mask = np.asarray(inputs["attn_mask"])
    padding_mask = np.asarray(inputs["padding_mask"])
    W = {n: np.asarray(inputs["W" + n], np.float32)
         for n in ["q", "k", "v", "kc", "vc", "g1", "g2", "p"]}
    bvec = {n: np.asarray(inputs["b" + n], np.float32)
            for n in ["q", "k", "v", "kc", "vc", "g1", "g2", "p"]}
    has_b = {n: bool(np.any(bvec[n] != 0)) for n in bvec}

    mask2 = np.asarray(attn_mask).reshape(T, T)
    order, slots, group_specs, kv_tiles = _plan(mask2)
    npart = sum(g[1] for g in group_specs if g[2] is not None)
    pslots = slots[:npart]

    nc = build_program(tuple(slots), tuple(group_specs), kv_tiles, has_b)

    KV = kv_tiles * KT
    w8 = {n: _dr8(W[n], WS) for n in ["q", "k", "kc"]}
    g12p = _chunk16(np.concatenate([W["g1"], W["g2"], W["p"]], axis=1))

    in_maps = []
    for core in range(8):
        b, j = divmod(core, 4)
        rows = core_rows(core, order)
        xT = x[b].T.astype(np.float32)            # [C, T]
        xq = np.ascontiguousarray(xT[:, rows])
        cTf = np.zeros((C, MP), np.float32)
        cTf[:, :M] = c[b].T
        pad = np.zeros((P, 1), np.float32)
        pad[:M, 0] = np.where(padding_mask[b] != 0, 0.0, NEG)
        a8 = np.concatenate([_dr8(xq), w8["q"], _dr8(cTf), w8["kc"]], axis=3)
        b8 = np.concatenate([_dr8(xT[:, :KV]), w8["k"]], axis=3)
        xv = np.concatenate([_chunk16(xT[:, :KV]), _chunk16(W["v"])], axis=2)
        cv = np.concatenate([_chunk16(cTf), _chunk16(W["vc"])], axis=2)
        im = {"a8": a8, "b8": b8, "xv": xv, "cv": cv, "g12p": g12p,
              "padb": pad}
        if npart:
            mm = np.zeros((P, npart * QP), np.float16)
            for nslot, (cc, s) in enumerate(pslots):
                qrows = rows[cc * QP:(cc + 1) * QP]
                blk = mask2[qrows][:, s * KT:(s + 1) * KT]  # [QP, KT]
                mm[:, nslot * QP:(nslot + 1) * QP] = np.where(blk.T, 1.0, 0.0)
            im["maskm"] = mm
        for n in ["q", "k", "kc", "g1", "g2"]:
            if has_b[n]:
                v = bvec[n] * (1.0 / np.sqrt(D) if n == "q" else 1.0)
                im["b" + n] = np.ascontiguousarray(
                    v.reshape(PAIRS, P).T).astype(np.float32)
        for n in ["v", "vc", "p"]:
            if has_b[n]:
                im["b" + n] = bvec[n].reshape(1, C).astype(np.float16)
        in_maps.append(im)
    return nc, in_maps


def kernel(**inputs):
    nc, in_maps = prepare(inputs)
    res = bass_utils.run_bass_kernel_spmd(nc, in_maps, core_ids=list(range(8)))
    mask2 = np.asarray(inputs["attn_mask"]).reshape(T, T)
    order = _plan(mask2)[0]
    out = np.empty((B, T, C), np.float32)
    for core in range(8):
        b = core // 4
        rows = core_rows(core, order)
        out[b, rows] = res.results[core]["out"].astype(np.float32)
    return out


# revision 22
# speedup vs baseline: 1.3542x; 1.1264x over previous
"""ConditionGateAttention Trainium2 kernel (v3).

Gated dual-attention block: causal self-attention + cross-attention to a
77-token condition, sigmoid cross-gating, output projection.

  B=2, T=2048, M=77, C=512, H=8 heads, D=64.

Sharding (8 cores): core = (b=core//4, j=core%4). Queries of batch b are
sorted by causal extent (host-side) and dealt round-robin to the 4 cores
in 8 "positions" of 64 queries each; position c needs keys only up to a
uniform extent (2(c+1) k-tiles for the causal mask), so every core does
the exact balanced share of causal work (144 k-tile units vs 192 for
contiguous-chunk sharding) with a program-uniform shape. K/V are computed
for the full batch locally (no collectives).

Precision: q/k/kc projections run in fp8e4(e4m3) DoubleRow mode (weights
scaled x16 on host to dodge fp8 subnormals; rescaled during the PSUM
eviction). Attention (QK/AV), v, gates and output projection stay fp16
(validated ~5.6e-3 rel err; fp8 probabilities/v would blow the 2e-2 gate).

Schedule: per (pair, head): cross-attention QK+exp leads (fills ACT while
PE zeroes/fills the self path), then 16-slot exp groups with a lag-2
QK->exp->AV software pipeline. Partially-masked slots are packed into the
leading group(s) (trailing for pair0-h0 so its AV can start before v-proj
of late k-tiles lands) and masked with one GPSIMD multiply. Denominators
ride AV as a ones-column on V; self-branch PSUM is evicted to SBUF
immediately (frees the PSUM buffer for the next head) and normalized via
reciprocal -> DMA partition-broadcast -> fused multiply on DVE. V/VC
PSUM evictions run on ACT (idle during the projection lead-in); q/k/kc
rescale-evictions on DVE; per-pair y tiles let the gate matmuls start
while the last pair is still normalizing.
"""
import numpy as np
import ml_dtypes
from contextlib import ExitStack

import concourse.bass as bass
import concourse.tile as tile
from concourse import bacc, mybir
from concourse import bass_utils

B, T, M, C, H = 2, 2048, 77, 512, 8
D = C // H            # 64
P = 128
KI = C // P           # 4 fp16 contraction chunks
KP = C // 256         # 2 fp8 DoubleRow contraction chunks (256 each)
PAIRS = H // 2        # pair i = heads 2i,2i+1 = C rows 128i..128i+128
NPOS = 8              # query positions per core
QP = 64               # queries per position
TQ = NPOS * QP        # 512 queries per core
KT = 128              # k-tile size
GS = 16               # slots per exp/psum group ([128, 1024] fp32 = 2 banks)
NEG = -30000.0
MP = 128              # condition length padded to 128
DA = D + 1            # V augmented with ones-column
WS = 16.0             # host-side fp8 weight scale

f8 = mybir.dt.float8e4
f16 = mybir.dt.float16
f32 = mybir.dt.float32
AF = mybir.ActivationFunctionType
ALU = mybir.AluOpType
DR = mybir.MatmulPerfMode.DoubleRow

_cache = {}


def build_program(slots, group_specs, kv_tiles, has_b):
    """slots: tuple of (pos, ktile), partial-masked slots first (uniform
    across cores). group_specs: tuple of (offset, size, mask_col | None).
    kv_tiles: number of 128-token k/v tiles to project."""
    key = (slots, group_specs, kv_tiles, tuple(sorted(has_b.items())))
    if key in _cache:
        return _cache[key]

    KV = kv_tiles * KT
    npart = sum(g[1] for g in group_specs if g[2] is not None)
    NG = len(group_specs)

    nc = bacc.Bacc("TRN2", num_devices=8, debug=False)

    # fused fp8 input: [xq8 | w8q | c8 | w8kc] then [w8k | x8]
    A0 = TQ + C + MP + C
    A1 = KV + C
    a8_d = nc.dram_tensor("a8", [P, KP, 2, A0], f8, kind="ExternalInput").ap()
    b8_d = nc.dram_tensor("b8", [P, KP, 2, A1], f8, kind="ExternalInput").ap()
    # fused fp16 input: [wv | xT] and [cT | wvc]
    xv_d = nc.dram_tensor("xv", [P, KI, KV + C], f16, kind="ExternalInput").ap()
    cv_d = nc.dram_tensor("cv", [P, KI, MP + C], f16, kind="ExternalInput").ap()
    g12p_d = nc.dram_tensor("g12p", [P, KI, 3 * C], f16, kind="ExternalInput").ap()
    if npart:
        mask_d = nc.dram_tensor("maskm", [P, npart * QP], f16, kind="ExternalInput").ap()
    pad_d = nc.dram_tensor("padb", [P, 1], f32, kind="ExternalInput").ap()
    bv_d = {}
    for n in ["q", "k", "kc", "g1", "g2"]:
        if has_b[n]:
            bv_d[n] = nc.dram_tensor(f"b{n}", [P, PAIRS], f32, kind="ExternalInput").ap()
    for n in ["v", "vc", "p"]:
        if has_b[n]:
            bv_d[n] = nc.dram_tensor(f"b{n}", [1, C], f16, kind="ExternalInput").ap()
    out_d = nc.dram_tensor("out", [TQ, C], f16, kind="ExternalOutput").ap()

    def emit(tc, ctx):
        consts = ctx.enter_context(tc.tile_pool(name="consts", bufs=1))
        acts = ctx.enter_context(tc.tile_pool(name="acts", bufs=1))
        work = ctx.enter_context(tc.tile_pool(name="work", bufs=4))
        nrm = ctx.enter_context(tc.tile_pool(name="nrm", bufs=4))
        ps_a = ctx.enter_context(tc.tile_pool(name="ps_a", bufs=2, space="PSUM"))
        ps_b = ctx.enter_context(tc.tile_pool(name="ps_b", bufs=2, space="PSUM"))
        ps_y = ctx.enter_context(tc.tile_pool(name="ps_y", bufs=2, space="PSUM"))

        # ---- input loads, split + ordered by first consumer ----
        a8 = consts.tile([P, KP, 2, A0], f8, name="a8")
        nc.sync.dma_start(a8[:, :, :, 0:TQ + C], a8_d[:, :, :, 0:TQ + C])
        nc.sync.dma_start(a8[:, :, :, TQ + C:], a8_d[:, :, :, TQ + C:])
        xq8 = a8[:, :, :, 0:TQ]
        w8q = a8[:, :, :, TQ:TQ + C]
        c8 = a8[:, :, :, TQ + C:TQ + C + MP]
        w8kc = a8[:, :, :, TQ + C + MP:]
        b8 = consts.tile([P, KP, 2, A1], f8, name="b8")
        bh = C + KV // 2
        nc.sync.dma_start(b8[:, :, :, 0:bh], b8_d[:, :, :, 0:bh])
        w8k = b8[:, :, :, 0:C]
        x8 = b8[:, :, :, C:]
        xv = consts.tile([P, KI, KV + C], f16, name="xv")
        xh1 = C + KV // 4
        xh2 = C + KV // 2
        nc.sync.dma_start(xv[:, :, 0:xh1], xv_d[:, :, 0:xh1])
        wv = xv[:, :, 0:C]
        xT = xv[:, :, C:]
        cv = consts.tile([P, KI, MP + C], f16, name="cv")
        nc.sync.dma_start(cv[:], cv_d)
        cT = cv[:, :, 0:MP]
        wvc = cv[:, :, MP:]
        nc.sync.dma_start(b8[:, :, :, bh:], b8_d[:, :, :, bh:])
        nc.sync.dma_start(xv[:, :, xh1:xh2], xv_d[:, :, xh1:xh2])
        nc.sync.dma_start(xv[:, :, xh2:], xv_d[:, :, xh2:])
        if npart:
            maskm = consts.tile([P, npart * QP], f16, name="maskm")
            nc.sync.dma_start(maskm[:], mask_d)
        pad_sb = consts.tile([P, 1], f32, name="padb")
        nc.sync.dma_start(pad_sb[:], pad_d)
        g12p = consts.tile([P, KI, 3 * C], f16, name="g12p")
        nc.sync.dma_start(g12p[:], g12p_d)
        wg = {"g1": g12p[:, :, 0:C], "g2": g12p[:, :, C:2 * C],
              "p": g12p[:, :, 2 * C:]}
        zcol = consts.tile([P, DA], f16, name="zcol")
        nc.vector.memset(zcol[:], 0.0)
        bv = {}
        for n, d in bv_d.items():
            if n in ("v", "vc", "p"):
                bv[n] = consts.tile([P, C], f16, name=f"b{n}")
                nc.sync.dma_start(bv[n][:],
                                  d[0:1, :].unsqueeze(1).to_broadcast((1, P, C)))
            else:
                bv[n] = consts.tile([P, PAIRS], f32, name=f"b{n}")
                nc.sync.dma_start(bv[n][:], d)

        # ---- persistent activation tiles ----
        q16 = acts.tile([P, PAIRS, TQ], f16, name="q16")
        k16 = acts.tile([P, PAIRS, KV], f16, name="k16")
        kc16 = acts.tile([P, PAIRS, MP], f16, name="kc16")
        v_sb = [acts.tile([P, H * DA], f16, name=f"v{m}") for m in range(kv_tiles)]
        vc_sb = acts.tile([P, H * DA], f16, name="vc")
        y16 = [acts.tile([P, TQ], f16, name=f"y16_{i}") for i in range(PAIRS)]
        yc16 = [acts.tile([P, TQ], f16, name=f"yc16_{i}") for i in range(PAIRS)]
        g1t = [acts.tile([P, TQ], f16, name=f"g1_{o}") for o in range(PAIRS)]
        g2t = [acts.tile([P, TQ], f16, name=f"g2_{o}") for o in range(PAIRS)]
        zt = [acts.tile([P, TQ], f16, name=f"z_{o}") for o in range(PAIRS)]

        # ---- fp8 DoubleRow projection ----
        def proj8(w8t, rhs8, n_free, dst, i, invs, bname):
            for t0 in range(0, n_free, 512):
                fw = min(512, n_free - t0)
                ps = ps_b.tile([P, fw], f32, tag="psb", name="pj")
                for tt in range(t0, t0 + fw, 256):
                    cw = min(256, t0 + fw - tt)
                    for kp in range(KP):
                        nc.tensor.matmul(
                            ps[:, tt - t0:tt - t0 + cw],
                            w8t[:, kp, :, P * i:P * i + P],
                            rhs8[:, kp, :, tt:tt + cw],
                            start=(kp == 0), stop=(kp == KP - 1),
                            perf_mode=DR)
                if has_b[bname]:
                    nc.scalar.activation(dst[:, i, t0:t0 + fw], ps[:],
                                         AF.Identity, bias=bv[bname][:, i:i + 1],
                                         scale=invs)
                else:
                    nc.vector.tensor_scalar_mul(dst[:, i, t0:t0 + fw],
                                                ps[:], invs)

        # ---- fp16 V projection (natural layout, ones-augmented) ----
        def vproj(wt, src, m, out_tile, ones_rows, bname):
            ps = ps_b.tile([P, 512], f32, tag="psb", name="pv")
            for ki in range(KI):
                nc.tensor.matmul(ps[:], src[:, ki, m * P:m * P + P],
                                 wt[:, ki, :], start=(ki == 0),
                                 stop=(ki == KI - 1))
            dst = out_tile.rearrange("p (h e) -> p h e", e=DA)
            if m % 2 == 0:
                nc.scalar.copy(dst[:, :, 0:D],
                               ps[:].rearrange("p (h e) -> p h e", e=D))
            else:
                nc.vector.tensor_copy(dst[:, :, 0:D],
                                      ps[:].rearrange("p (h e) -> p h e", e=D))
            if has_b[bname]:
                nc.vector.tensor_tensor(
                    dst[:, :, 0:D], dst[:, :, 0:D],
                    bv[bname][:].rearrange("p (h e) -> p h e", e=D), ALU.add)
            if ones_rows < P:
                nc.gpsimd.memset(dst[:, :, D:DA], 0.0)
            nc.gpsimd.memset(dst[0:ones_rows, :, D:DA], 1.0)

        invq = 1.0 / (WS * float(np.sqrt(D)))
        invk = 1.0 / WS

        def kproj(i):
            proj8(w8k, x8, KV, k16, i, invk, "k")

        # ---- attention for (pair i, head h) ----
        def attention(i, h, extra=None, fulls_first=False):
            b0 = h * D
            hcol = (2 * i + h) * DA

            # cross-attention leads: fills ACT while the self path spins up
            scp = ps_b.tile([P, TQ], f32, tag="psb", name="scp")
            nc.tensor.matmul(scp[:], kc16[b0:b0 + D, i, :],
                             q16[b0:b0 + D, i, :], start=True, stop=True)
            pct = work.tile([P, TQ], f16, tag="pt", name="pct")
            nc.scalar.activation(pct[:], scp[:], AF.Exp, bias=pad_sb[:, 0:1])
            if extra is not None:
                extra()  # next pair's k-projection, off the critical path

            # self-attention: zero the accumulator (start-bit zeroing can't
            # express 8 interleaved per-position windows), then lag-2
            # QK -> exp -> AV pipeline over the slot groups.
            yps = ps_y.tile([DA, TQ], f32, tag="y", name="yps")
            nc.tensor.matmul(yps[:], zcol[:], q16[:, i, :],
                             start=True, stop=False, skip_group_check=True)

            order = list(range(NG))
            if fulls_first:
                order = [g for g in order if group_specs[g][2] is None] + \
                        [g for g in order if group_specs[g][2] is not None]
            lastmap = {}
            for g in ([g for g in order if group_specs[g][2] is None] +
                      [g for g in order if group_specs[g][2] is not None]):
                off, size, _ = group_specs[g]
                for n in range(size):
                    lastmap[slots[off + n][0]] = off + n

            def qk_group(g):
                off, size, mcol = group_specs[g]
                lg = ps_a.tile([P, size * QP], f32, tag="lg", name="lg")
                for n in range(size):
                    cc, s = slots[off + n]
                    nc.tensor.matmul(
                        lg[:, n * QP:(n + 1) * QP],
                        k16[b0:b0 + D, i, s * KT:(s + 1) * KT],
                        q16[b0:b0 + D, i, cc * QP:(cc + 1) * QP],
                        start=True, stop=True)
                pt = work.tile([P, size * QP], f16, tag="pt", name="pt")
                nc.scalar.activation(pt[:], lg[:], AF.Exp)
                if mcol is not None:
                    nc.gpsimd.tensor_tensor(
                        pt[:], pt[:], maskm[:, mcol:mcol + size * QP], ALU.mult)
                return pt

            def av_group(g, pt):
                off, size, _ = group_specs[g]
                for n in range(size):
                    cc, s = slots[off + n]
                    nc.tensor.matmul(
                        yps[:, cc * QP:(cc + 1) * QP],
                        v_sb[s][:, hcol:hcol + DA],
                        pt[:, n * QP:(n + 1) * QP],
                        start=False, stop=(off + n == lastmap[cc]),
                        skip_group_check=True)

            # masked groups' AV runs last: the exp -> GPSIMD-mask chain then
            # has the whole head to complete instead of sitting at lag-2.
            pts = {}
            done = 0
            unmasked = [g for g in order if group_specs[g][2] is None]
            for gi, g in enumerate(order):
                pts[g] = qk_group(g)
                if gi == 0:
                    # cross AV: pct is ready by now; separate PSUM tile
                    ycps = ps_y.tile([DA, TQ], f32, tag="y", name="ycps")
                    nc.tensor.matmul(ycps[:], vc_sb[:, hcol:hcol + DA],
                                     pct[:], start=True, stop=True)
                    # cross normalize (PSUM-direct; frees ycps mid-head)
                    norm_branch(ycps, yc16[i], b0, evict=False)
                while done + 2 <= gi + 1 and done < len(unmasked):
                    av_group(unmasked[done], pts.pop(unmasked[done]))
                    done += 1
            for g in unmasked[done:]:
                av_group(g, pts.pop(g))
            for g in order:
                if g in pts:
                    av_group(g, pts.pop(g))

            # self normalize: evict PSUM immediately so the next head's
            # accumulator allocation doesn't wait on the normalize chain.
            norm_branch(yps, y16[i], b0, evict=True)

        def norm_branch(ps, dst, b0, evict):
            if evict:
                yraw = nrm.tile([DA, TQ], f32, tag="yraw")
                nc.vector.tensor_copy(yraw[:], ps[:])
                src = yraw
            else:
                src = ps
            rec = nrm.tile([1, TQ], f32, tag="rec")
            nc.vector.reciprocal(rec[:], src[D:DA, :])
            bc = nrm.tile([D, TQ], f32, tag="bc")
            nc.sync.dma_start(
                bc[:], rec[0:1, :].unsqueeze(1).to_broadcast((1, D, TQ)))
            nc.vector.tensor_tensor(dst[b0:b0 + D, :], src[0:D, :],
                                    bc[:], ALU.mult)

        # ---- gates, combine, output projection ----
        def gates_out():
            for o in range(PAIRS):
                for wname, srcs, dstt, bn in (("g1", y16, g1t, "g1"),
                                              ("g2", yc16, g2t, "g2")):
                    # alternate PSUM pools: ps_a/ps_y are idle by the tail,
                    # so 4 gate accumulations can be in flight instead of 2
                    pool = (ps_b, ps_y)[o % 2]
                    ps = pool.tile([P, TQ], f32,
                                   tag=("psb", "y")[o % 2], name="pg")
                    for i in range(PAIRS):
                        nc.tensor.matmul(ps[:], wg[wname][:, i, P * o:P * o + P],
                                         srcs[i][:], start=(i == 0),
                                         stop=(i == PAIRS - 1))
                    bias = bv[bn][:, o:o + 1] if has_b[bn] else 0.0
                    nc.scalar.activation(dstt[o][:], ps[:], AF.Sigmoid, bias=bias)
                t1 = work.tile([P, TQ], f16, tag="zt")
                nc.vector.tensor_tensor(t1[:], g1t[o][:], yc16[o][:], ALU.mult)
                nc.vector.tensor_tensor(zt[o][:], g2t[o][:], y16[o][:], ALU.mult)
                nc.vector.tensor_tensor(zt[o][:], zt[o][:], t1[:], ALU.add)
            for m in range(PAIRS):
                pool = (ps_b, ps_y)[m % 2]
                ps = pool.tile([P, C], f32, tag=("psb", "y")[m % 2], name="po")
                for o in range(PAIRS):
                    nc.tensor.matmul(ps[:], zt[o][:, P * m:P * m + P],
                                     wg["p"][:, o, :], start=(o == 0),
                                     stop=(o == PAIRS - 1))
                osb = work.tile([P, C], f16, tag="osb")
                if has_b["p"]:
                    nc.vector.tensor_tensor(osb[:], ps[:], bv["p"][:], ALU.add)
                elif m % 2 == 0:
                    nc.scalar.copy(osb[:], ps[:])
                else:
                    nc.vector.tensor_copy(osb[:], ps[:])
                nc.sync.dma_start(out_d[P * m:P * m + P, :], osb[:])

        # ---- schedule ----
        for i in range(PAIRS):
            proj8(w8q, xq8, TQ, q16, i, invq, "q")
        for i in range(PAIRS):
            proj8(w8kc, c8, MP, kc16, i, invk, "kc")
        kproj(0)
        vproj(wvc, cT, 0, vc_sb, M, "vc")
        for m in range(kv_tiles):
            vproj(wv, xT, m, v_sb[m], P, "v")
        for i in range(PAIRS):
            extra = (lambda i=i: kproj(i + 1)) if i + 1 < PAIRS else None
            attention(i, 0, extra=extra, fulls_first=(i == 0))
            attention(i, 1)
        gates_out()

    with tile.TileContext(nc) as tc, ExitStack() as ctx:
        emit(tc, ctx)
    nc.compile()
    _cache[key] = nc
    return nc


def _plan(mask2):
    """Derive the uniform attention schedule from the (shared) attn mask."""
    vis = mask2 != 0
    need = np.where(vis.any(1), vis.shape[1] - np.argmax(vis[:, ::-1], 1), 1)
    order = np.argsort(need, kind="stable")
    E, fulls, parts = [], [], []
    for c in range(NPOS):
        qc = order[4 * QP * c:4 * QP * (c + 1)]
        e = -(-int(need[qc].max()) // KT)
        E.append(e)
        f, p = [], []
        for s in range(e):
            blk = vis[qc][:, s * KT:(s + 1) * KT]
            (f if blk.all() else p).append(s)
        fulls.append(f)
        parts.append(p)
    pslots = [(c, s) for c in range(NPOS) for s in parts[c]]
    fslots = [(c, s) for c in range(NPOS) for s in fulls[c]]
    slots = pslots + fslots
    group_specs = []
    for o in range(0, len(pslots), GS):
        group_specs.append((o, min(GS, len(pslots) - o), o * QP))
    for o in range(0, len(fslots), GS):
        group_specs.append((len(pslots) + o, min(GS, len(fslots) - o), None))
    kv_tiles = max(E)
    return order, slots, group_specs, kv_tiles


def core_rows(core, order=None):
    """Global query indices handled by `core`, in output-row order."""
    if order is None:
        order = _plan(np.tril(np.ones((T, T), np.int64)))[0]
    j = core % 4
    return np.concatenate([order[4 * QP * c + j:4 * QP * (c + 1):4]
                           for c in range(NPOS)])


def _dr8(a, scale=1.0):
    # [C, N] f32 -> [128, KP, 2, N] fp8 DoubleRow layout
    Cr, N = a.shape
    return np.ascontiguousarray(
        (a * scale).reshape(KP, 2, P, N).transpose(2, 0, 1, 3)
    ).astype(ml_dtypes.float8_e4m3)


def _chunk16(a):
    # [C, N] f32 -> [128, KI, N] fp16
    Cr, N = a.shape
    return np.ascontiguousarray(
        a.reshape(KI, P, N).transpose(1, 0, 2)).astype(np.float16)


def prepare(inputs):
    x = np.asarray(inputs["x"], np.float32)
    c = np.asarray(inputs["c"], np.float32)
    attn_mask = np.asarray(inputs["attn_mask"])
    padding_mask = np.asarray(inputs["padding_mask"])
    W = {n: np.asarray(inputs["W" + n], np.float32)
         for n in ["q", "k", "v", "kc", "vc", "g1", "g2", "p"]}
    bvec = {n: np.asarray(inputs["b" + n], np.float32)
            for n in ["q", "k", "v", "kc", "vc", "g1", "g2", "p"]}
    has_b = {n: bool(np.any(bvec[n] != 0)) for n in bvec}

    mask2 = np.asarray(attn_mask).reshape(T, T)
    order, slots, group_specs, kv_tiles = _plan(mask2)
    npart = sum(g[1] for g in group_specs if g[2] is not None)
    pslots = slots[:npart]

    nc = build_program(tuple(slots), tuple(group_specs), kv_tiles, has_b)

    KV = kv_tiles * KT
    w8 = {n: _dr8(W[n], WS) for n in ["q", "k", "kc"]}
    g12p = _chunk16(np.concatenate([W["g1"], W["g2"], W["p"]], axis=1))

    in_maps = []
    for core in range(8):
        b, j = divmod(core, 4)
        rows = core_rows(core, order)
        xT = x[b].T.astype(np.float32)            # [C, T]
        xq = np.ascontiguousarray(xT[:, rows])
        cTf = np.zeros((C, MP), np.float32)
        cTf[:, :M] = c[b].T
        pad = np.zeros((P, 1), np.float32)
        pad[:M, 0] = np.where(padding_mask[b] != 0, 0.0, NEG)
        a8 = np.concatenate([_dr8(xq), w8["q"], _dr8(cTf), w8["kc"]], axis=3)
        b8 = np.concatenate([w8["k"], _dr8(xT[:, :KV])], axis=3)
        xv = np.concatenate([_chunk16(W["v"]), _chunk16(xT[:, :KV])], axis=2)
        cv = np.concatenate([_chunk16(cTf), _chunk16(W["vc"])], axis=2)
        im = {"a8": a8, "b8": b8, "xv": xv, "cv": cv, "g12p": g12p,
              "padb": pad}
        if npart:
            mm = np.zeros((P, npart * QP), np.float16)
            for nslot, (cc, s) in enumerate(pslots):
                qrows = rows[cc * QP:(cc + 1) * QP]
                blk = mask2[qrows][:, s * KT:(s + 1) * KT]  # [QP, KT]
                mm[:, nslot * QP:(nslot + 1) * QP] = np.where(blk.T, 1.0, 0.0)
            im["maskm"] = mm
        for n in ["q", "k", "kc", "g1", "g2"]:
            if has_b[n]:
                v = bvec[n] * (1.0 / np.sqrt(D) if n == "q" else 1.0)
                im["b" + n] = np.ascontiguousarray(
                    v.reshape(PAIRS, P).T).astype(np.float32)
        for n in ["v", "vc", "p"]:
            if has_b[n]:
                im["b" + n] = bvec[n].reshape(1, C).astype(np.float16)
        in_maps.append(im)
    return nc, in_maps


def kernel(**inputs):
    nc, in_maps = prepare(inputs)
    res = bass_utils.run_bass_kernel_spmd(nc, in_maps, core_ids=list(range(8)))
    mask2 = np.asarray(inputs["attn_mask"]).reshape(T, T)
    order = _plan(mask2)[0]
    out = np.empty((B, T, C), np.float32)
    for core in range(8):
        b = core // 4
        rows = core_rows(core, order)
        out[b, rows] = res.results[core]["out"].astype(np.float32)
    return out


# revision 28
# speedup vs baseline: 1.3823x; 1.0207x over previous
"""ConditionGateAttention Trainium2 kernel (v3).

Gated dual-attention block: causal self-attention + cross-attention to a
77-token condition, sigmoid cross-gating, output projection.

  B=2, T=2048, M=77, C=512, H=8 heads, D=64.

Sharding (8 cores): core = (b=core//4, j=core%4). Queries of batch b are
sorted by causal extent (host-side) and dealt round-robin to the 4 cores
in 8 "positions" of 64 queries each; position c needs keys only up to a
uniform extent (2(c+1) k-tiles for the causal mask), so every core does
the exact balanced share of causal work (144 k-tile units vs 192 for
contiguous-chunk sharding) with a program-uniform shape. K/V are computed
for the full batch locally (no collectives).

Precision: q/k/kc projections run in fp8e4(e4m3) DoubleRow mode (weights
scaled x16 on host to dodge fp8 subnormals; rescaled during the PSUM
eviction). Attention (QK/AV), v, gates and output projection stay fp16
(validated ~5.6e-3 rel err; fp8 probabilities/v would blow the 2e-2 gate).

Schedule: per (pair, head): cross-attention QK+exp leads (fills ACT while
PE zeroes/fills the self path), then 16-slot exp groups with a lag-2
QK->exp->AV software pipeline. Partially-masked slots are packed into the
leading group(s) (trailing for pair0-h0 so its AV can start before v-proj
of late k-tiles lands) and masked with one GPSIMD multiply. Denominators
ride AV as a ones-column on V; self-branch PSUM is evicted to SBUF
immediately (frees the PSUM buffer for the next head) and normalized via
reciprocal -> DMA partition-broadcast -> fused multiply on DVE. V/VC
PSUM evictions run on ACT (idle during the projection lead-in); q/k/kc
rescale-evictions on DVE; per-pair y tiles let the gate matmuls start
while the last pair is still normalizing.
"""
import numpy as np
import ml_dtypes
from contextlib import ExitStack

import concourse.bass as bass
import concourse.tile as tile
from concourse import bacc, mybir
from concourse import bass_utils

B, T, M, C, H = 2, 2048, 77, 512, 8
D = C // H            # 64
P = 128
KI = C // P           # 4 fp16 contraction chunks
KP = C // 256         # 2 fp8 DoubleRow contraction chunks (256 each)
PAIRS = H // 2        # pair i = heads 2i,2i+1 = C rows 128i..128i+128
NPOS = 8              # query positions per core
QP = 64               # queries per position
TQ = NPOS * QP        # 512 queries per core
KT = 128              # k-tile size
GS = 16               # slots per exp/psum group ([128, 1024] fp32 = 2 banks)
NEG = -30000.0
MP = 128              # condition length padded to 128
DA = D + 1            # V augmented with ones-column
WS = 16.0             # host-side fp8 weight scale

f8 = mybir.dt.float8e4
f16 = mybir.dt.float16
f32 = mybir.dt.float32
AF = mybir.ActivationFunctionType
ALU = mybir.AluOpType
DR = mybir.MatmulPerfMode.DoubleRow

_cache = {}


def build_program(slots, group_specs, kv_tiles, has_b):
    """slots: tuple of (pos, ktile), partial-masked slots first (uniform
    across cores). group_specs: tuple of (offset, size, mask_col | None).
    kv_tiles: number of 128-token k/v tiles to project."""
    key = (slots, group_specs, kv_tiles, tuple(sorted(has_b.items())))
    if key in _cache:
        return _cache[key]

    KV = kv_tiles * KT
    npart = sum(g[1] for g in group_specs if g[2] is not None)
    NG = len(group_specs)

    nc = bacc.Bacc("TRN2", num_devices=8, debug=False)

    # fused fp8 input: [xq8 | w8q | c8 | w8kc] then [w8k | x8]
    A0 = TQ + C + MP + C
    A1 = KV + C
    a8_d = nc.dram_tensor("a8", [P, KP, 2, A0], f8, kind="ExternalInput").ap()
    b8_d = nc.dram_tensor("b8", [P, KP, 2, A1], f8, kind="ExternalInput").ap()
    # fused fp16 input: [wv | xT] and [cT | wvc]
    xv_d = nc.dram_tensor("xv", [P, KI, KV + C], f16, kind="ExternalInput").ap()
    cv_d = nc.dram_tensor("cv", [P, KI, MP + C], f16, kind="ExternalInput").ap()
    g12p_d = nc.dram_tensor("g12p", [P, KI, 3 * C], f16, kind="ExternalInput").ap()
    if npart:
        mask_d = nc.dram_tensor("maskm", [P, npart * QP], f16, kind="ExternalInput").ap()
    pad_d = nc.dram_tensor("padb", [P, 1], f32, kind="ExternalInput").ap()
    bv_d = {}
    for n in ["q", "k", "kc", "g1", "g2"]:
        if has_b[n]:
            bv_d[n] = nc.dram_tensor(f"b{n}", [P, PAIRS], f32, kind="ExternalInput").ap()
    for n in ["v", "vc", "p"]:
        if has_b[n]:
            bv_d[n] = nc.dram_tensor(f"b{n}", [1, C], f16, kind="ExternalInput").ap()
    out_d = nc.dram_tensor("out", [TQ, C], f16, kind="ExternalOutput").ap()

    def emit(tc, ctx):
        consts = ctx.enter_context(tc.tile_pool(name="consts", bufs=1))
        acts = ctx.enter_context(tc.tile_pool(name="acts", bufs=1))
        work = ctx.enter_context(tc.tile_pool(name="work", bufs=4))
        nrm = ctx.enter_context(tc.tile_pool(name="nrm", bufs=4))
        ps_a = ctx.enter_context(tc.tile_pool(name="ps_a", bufs=2, space="PSUM"))
        ps_b = ctx.enter_context(tc.tile_pool(name="ps_b", bufs=2, space="PSUM"))
        ps_y = ctx.enter_context(tc.tile_pool(name="ps_y", bufs=2, space="PSUM"))

        # ---- input loads, split + ordered by first consumer ----
        a8 = consts.tile([P, KP, 2, A0], f8, name="a8")
        nc.sync.dma_start(a8[:, :, :, 0:TQ + C], a8_d[:, :, :, 0:TQ + C])
        nc.sync.dma_start(a8[:, :, :, TQ + C:], a8_d[:, :, :, TQ + C:])
        xq8 = a8[:, :, :, 0:TQ]
        w8q = a8[:, :, :, TQ:TQ + C]
        c8 = a8[:, :, :, TQ + C:TQ + C + MP]
        w8kc = a8[:, :, :, TQ + C + MP:]
        b8 = consts.tile([P, KP, 2, A1], f8, name="b8")
        bh = C + KV // 2
        nc.sync.dma_start(b8[:, :, :, 0:bh], b8_d[:, :, :, 0:bh])
        w8k = b8[:, :, :, 0:C]
        x8 = b8[:, :, :, C:]
        cv = consts.tile([P, KI, MP + C], f16, name="cv")
        nc.sync.dma_start(cv[:], cv_d)
        cT = cv[:, :, 0:MP]
        wvc = cv[:, :, MP:]
        xv = consts.tile([P, KI, KV + C], f16, name="xv")
        xh1 = C + KV // 4
        xh2 = C + KV // 2
        nc.sync.dma_start(xv[:, :, 0:xh1], xv_d[:, :, 0:xh1])
        wv = xv[:, :, 0:C]
        xT = xv[:, :, C:]
        nc.sync.dma_start(b8[:, :, :, bh:], b8_d[:, :, :, bh:])
        nc.sync.dma_start(xv[:, :, xh1:xh2], xv_d[:, :, xh1:xh2])
        nc.sync.dma_start(xv[:, :, xh2:], xv_d[:, :, xh2:])
        if npart:
            maskm = consts.tile([P, npart * QP], f16, name="maskm")
            nc.sync.dma_start(maskm[:], mask_d)
        pad_sb = consts.tile([P, 1], f32, name="padb")
        nc.sync.dma_start(pad_sb[:], pad_d)
        g12p = consts.tile([P, KI, 3 * C], f16, name="g12p")
        nc.sync.dma_start(g12p[:], g12p_d)
        wg = {"g1": g12p[:, :, 0:C], "g2": g12p[:, :, C:2 * C],
              "p": g12p[:, :, 2 * C:]}
        zcol = consts.tile([P, DA], f16, name="zcol")
        nc.vector.memset(zcol[:], 0.0)
        bv = {}
        for n, d in bv_d.items():
            if n in ("v", "vc", "p"):
                bv[n] = consts.tile([P, C], f16, name=f"b{n}")
                nc.sync.dma_start(bv[n][:],
                                  d[0:1, :].unsqueeze(1).to_broadcast((1, P, C)))
            else:
                bv[n] = consts.tile([P, PAIRS], f32, name=f"b{n}")
                nc.sync.dma_start(bv[n][:], d)

        # ---- persistent activation tiles ----
        q16 = acts.tile([P, PAIRS, TQ], f16, name="q16")
        k16 = acts.tile([P, PAIRS, KV], f16, name="k16")
        kc16 = acts.tile([P, PAIRS, MP], f16, name="kc16")
        v_sb = [acts.tile([P, H * DA], f16, name=f"v{m}") for m in range(kv_tiles)]
        vc_sb = acts.tile([P, H * DA], f16, name="vc")
        y16 = [acts.tile([P, TQ], f16, name=f"y16_{i}") for i in range(PAIRS)]
        yc16 = [acts.tile([P, TQ], f16, name=f"yc16_{i}") for i in range(PAIRS)]
        g1t = [acts.tile([P, TQ], f16, name=f"g1_{o}") for o in range(PAIRS)]
        g2t = [acts.tile([P, TQ], f16, name=f"g2_{o}") for o in range(PAIRS)]
        zt = [acts.tile([P, TQ], f16, name=f"z_{o}") for o in range(PAIRS)]

        # ---- fp8 DoubleRow projection ----
        def proj8_tile(w8t, rhs8, t0, fw, dst, i, invs, bname):
            ps = ps_b.tile([P, fw], f32, tag="psb", name="pj")
            for tt in range(t0, t0 + fw, 256):
                cw = min(256, t0 + fw - tt)
                for kp in range(KP):
                    nc.tensor.matmul(
                        ps[:, tt - t0:tt - t0 + cw],
                        w8t[:, kp, :, P * i:P * i + P],
                        rhs8[:, kp, :, tt:tt + cw],
                        start=(kp == 0), stop=(kp == KP - 1),
                        perf_mode=DR)
            if has_b[bname]:
                nc.scalar.activation(dst[:, i, t0:t0 + fw], ps[:],
                                     AF.Identity, bias=bv[bname][:, i:i + 1],
                                     scale=invs)
            else:
                nc.vector.tensor_scalar_mul(dst[:, i, t0:t0 + fw],
                                            ps[:], invs)

        def proj8(w8t, rhs8, n_free, dst, i, invs, bname):
            for t0 in range(0, n_free, 512):
                proj8_tile(w8t, rhs8, t0, min(512, n_free - t0),
                           dst, i, invs, bname)

        # ---- fp16 V projection (natural layout, ones-augmented) ----
        def vproj(wt, src, m, out_tile, ones_rows, bname):
            ps = ps_b.tile([P, 512], f32, tag="psb", name="pv")
            for ki in range(KI):
                nc.tensor.matmul(ps[:], src[:, ki, m * P:m * P + P],
                                 wt[:, ki, :], start=(ki == 0),
                                 stop=(ki == KI - 1))
            dst = out_tile.rearrange("p (h e) -> p h e", e=DA)
            if m % 2 == 0:
                nc.scalar.copy(dst[:, :, 0:D],
                               ps[:].rearrange("p (h e) -> p h e", e=D))
            else:
                nc.vector.tensor_copy(dst[:, :, 0:D],
                                      ps[:].rearrange("p (h e) -> p h e", e=D))
            if has_b[bname]:
                nc.vector.tensor_tensor(
                    dst[:, :, 0:D], dst[:, :, 0:D],
                    bv[bname][:].rearrange("p (h e) -> p h e", e=D), ALU.add)
            if ones_rows < P:
                nc.gpsimd.memset(dst[:, :, D:DA], 0.0)
            nc.gpsimd.memset(dst[0:ones_rows, :, D:DA], 1.0)

        invq = 1.0 / (WS * float(np.sqrt(D)))
        invk = 1.0 / WS

        def kproj(i, lo=0, hi=KV):
            for t0 in range(lo, hi, 512):
                proj8_tile(w8k, x8, t0, min(512, hi - t0), k16, i, invk, "k")

        # ---- attention for (pair i, head h) ----
        def attention(i, h, extra=None, fulls_first=False):
            b0 = h * D
            hcol = (2 * i + h) * DA

            # cross-attention leads: fills ACT while the self path spins up
            scp = ps_b.tile([P, TQ], f32, tag="psb", name="scp")
            nc.tensor.matmul(scp[:], kc16[b0:b0 + D, i, :],
                             q16[b0:b0 + D, i, :], start=True, stop=True)
            pct = work.tile([P, TQ], f16, tag="pt", name="pct")
            nc.scalar.activation(pct[:], scp[:], AF.Exp, bias=pad_sb[:, 0:1])
            if extra is not None:
                extra()  # next pair's k-projection, off the critical path

            # self-attention: zero the accumulator (start-bit zeroing can't
            # express 8 interleaved per-position windows), then lag-2
            # QK -> exp -> AV pipeline over the slot groups.
            yps = ps_y.tile([DA, TQ], f32, tag="y", name="yps")
            nc.tensor.matmul(yps[:], zcol[:], q16[:, i, :],
                             start=True, stop=False, skip_group_check=True)

            order = list(range(NG))
            if fulls_first:
                order = [g for g in order if group_specs[g][2] is None] + \
                        [g for g in order if group_specs[g][2] is not None]
            lastmap = {}
            for g in ([g for g in order if group_specs[g][2] is None] +
                      [g for g in order if group_specs[g][2] is not None]):
                off, size, _ = group_specs[g]
                for n in range(size):
                    lastmap[slots[off + n][0]] = off + n

            def qk_group(g):
                off, size, mcol = group_specs[g]
                lg = ps_a.tile([P, size * QP], f32, tag="lg", name="lg")
                for n in range(size):
                    cc, s = slots[off + n]
                    nc.tensor.matmul(
                        lg[:, n * QP:(n + 1) * QP],
                        k16[b0:b0 + D, i, s * KT:(s + 1) * KT],
                        q16[b0:b0 + D, i, cc * QP:(cc + 1) * QP],
                        start=True, stop=True)
                pt = work.tile([P, size * QP], f16, tag="pt", name="pt")
                nc.scalar.activation(pt[:], lg[:], AF.Exp)
                if mcol is not None:
                    nc.vector.tensor_tensor(
                        pt[:], pt[:], maskm[:, mcol:mcol + size * QP], ALU.mult)
                return pt

            def av_group(g, pt):
                off, size, _ = group_specs[g]
                for n in range(size):
                    cc, s = slots[off + n]
                    nc.tensor.matmul(
                        yps[:, cc * QP:(cc + 1) * QP],
                        v_sb[s][:, hcol:hcol + DA],
                        pt[:, n * QP:(n + 1) * QP],
                        start=False, stop=(off + n == lastmap[cc]),
                        skip_group_check=True)

            # masked groups' AV runs last: the exp -> GPSIMD-mask chain then
            # has the whole head to complete instead of sitting at lag-2.
            pts = {}
            done = 0
            unmasked = [g for g in order if group_specs[g][2] is None]
            for gi, g in enumerate(order):
                pts[g] = qk_group(g)
                if gi == 0:
                    # cross AV: pct is ready by now; ps_b so ps_y stays a
                    # dedicated 2-deep self-accumulator rotation
                    ycps = ps_b.tile([DA, TQ], f32, tag="psb", name="ycps")
                    nc.tensor.matmul(ycps[:], vc_sb[:, hcol:hcol + DA],
                                     pct[:], start=True, stop=True)
                    # cross normalize (PSUM-direct; frees ycps mid-head)
                    norm_branch(ycps, yc16[i], b0, evict=False)
                while done + 2 <= gi + 1 and done < len(unmasked):
                    av_group(unmasked[done], pts.pop(unmasked[done]))
                    done += 1
            for g in unmasked[done:]:
                av_group(g, pts.pop(g))
            for g in order:
                if g in pts:
                    av_group(g, pts.pop(g))

            # self normalize: evict PSUM immediately so the next head's
            # accumulator allocation doesn't wait on the normalize chain.
            norm_branch(yps, y16[i], b0, evict=True)

        def norm_branch(ps, dst, b0, evict):
            if evict:
                yraw = nrm.tile([DA, TQ], f32, tag="yraw")
                nc.vector.tensor_copy(yraw[:], ps[:])
                src = yraw
            else:
                src = ps
            rec = nrm.tile([1, TQ], f32, tag="rec")
            nc.vector.reciprocal(rec[:], src[D:DA, :])
            bc = nrm.tile([D, TQ], f32, tag="bc")
            nc.sync.dma_start(
                bc[:], rec[0:1, :].unsqueeze(1).to_broadcast((1, D, TQ)))
            nc.vector.tensor_tensor(dst[b0:b0 + D, :], src[0:D, :],
                                    bc[:], ALU.mult)

        # ---- gates, combine, output projection ----
        def gates_out():
            for o in range(PAIRS):
                for wname, srcs, dstt, bn in (("g1", y16, g1t, "g1"),
                                              ("g2", yc16, g2t, "g2")):
                    # alternate PSUM pools: ps_a/ps_y are idle by the tail,
                    # so 4 gate accumulations can be in flight instead of 2
                    pool = (ps_b, ps_y)[o % 2]
                    ps = pool.tile([P, TQ], f32,
                                   tag=("psb", "y")[o % 2], name="pg")
                    for i in range(PAIRS):
                        nc.tensor.matmul(ps[:], wg[wname][:, i, P * o:P * o + P],
                                         srcs[i][:], start=(i == 0),
                                         stop=(i == PAIRS - 1))
                    bias = bv[bn][:, o:o + 1] if has_b[bn] else 0.0
                    nc.scalar.activation(dstt[o][:], ps[:], AF.Sigmoid, bias=bias)
                t1 = work.tile([P, TQ], f16, tag="zt")
                nc.vector.tensor_tensor(t1[:], g1t[o][:], yc16[o][:], ALU.mult)
                nc.vector.tensor_tensor(zt[o][:], g2t[o][:], y16[o][:], ALU.mult)
                nc.vector.tensor_tensor(zt[o][:], zt[o][:], t1[:], ALU.add)
            for m in range(PAIRS):
                pool = (ps_b, ps_y)[m % 2]
                ps = pool.tile([P, C], f32, tag=("psb", "y")[m % 2], name="po")
                for o in range(PAIRS):
                    nc.tensor.matmul(ps[:], zt[o][:, P * m:P * m + P],
                                     wg["p"][:, o, :], start=(o == 0),
                                     stop=(o == PAIRS - 1))
                osb = work.tile([P, C], f16, tag="osb")
                if has_b["p"]:
                    nc.vector.tensor_tensor(osb[:], ps[:], bv["p"][:], ALU.add)
                elif m % 2 == 0:
                    nc.scalar.copy(osb[:], ps[:])
                else:
                    nc.vector.tensor_copy(osb[:], ps[:])
                nc.sync.dma_start(out_d[P * m:P * m + P, :], osb[:])

        # ---- schedule ----
        for i in range(PAIRS):
            proj8(w8q, xq8, TQ, q16, i, invq, "q")
        for i in range(PAIRS):
            proj8(w8kc, c8, MP, kc16, i, invk, "kc")
        kproj(0, 0, KV // 2)
        vproj(wvc, cT, 0, vc_sb, M, "vc")
        for m in range(kv_tiles // 2):
            vproj(wv, xT, m, v_sb[m], P, "v")
        kproj(0, KV // 2, KV)
        for m in range(kv_tiles // 2, kv_tiles):
            vproj(wv, xT, m, v_sb[m], P, "v")
        for i in range(PAIRS):
            extra = (lambda i=i: kproj(i + 1)) if i + 1 < PAIRS else None
            attention(i, 0, extra=extra, fulls_first=(i == 0))
            attention(i, 1)
        gates_out()

    with tile.TileContext(nc) as tc, ExitStack() as ctx:
        emit(tc, ctx)
    nc.compile()
    _cache[key] = nc
    return nc


def _plan(mask2):
    """Derive the uniform attention schedule from the (shared) attn mask."""
    vis = mask2 != 0
    need = np.where(vis.any(1), vis.shape[1] - np.argmax(vis[:, ::-1], 1), 1)
    order = np.argsort(need, kind="stable")
    E, fulls, parts = [], [], []
    for c in range(NPOS):
        qc = order[4 * QP * c:4 * QP * (c + 1)]
        e = -(-int(need[qc].max()) // KT)
        E.append(e)
        f, p = [], []
        for s in range(e):
            blk = vis[qc][:, s * KT:(s + 1) * KT]
            (f if blk.all() else p).append(s)
        fulls.append(f)
        parts.append(p)
    pslots = [(c, s) for c in range(NPOS) for s in parts[c]]
    fslots = [(c, s) for c in range(NPOS) for s in fulls[c]]
    slots = pslots + fslots
    group_specs = []
    for o in range(0, len(pslots), GS):
        group_specs.append((o, min(GS, len(pslots) - o), o * QP))
    for o in range(0, len(fslots), GS):
        group_specs.append((len(pslots) + o, min(GS, len(fslots) - o), None))
    kv_tiles = max(E)
    return order, slots, group_specs, kv_tiles


def core_rows(core, order=None):
    """Global query indices handled by `core`, in output-row order."""
    if order is None:
        order = _plan(np.tril(np.ones((T, T), np.int64)))[0]
    j = core % 4
    return np.concatenate([order[4 * QP * c + j:4 * QP * (c + 1):4]
                           for c in range(NPOS)])


def _dr8(a, scale=1.0):
    # [C, N] f32 -> [128, KP, 2, N] fp8 DoubleRow layout
    Cr, N = a.shape
    return np.ascontiguousarray(
        (a * scale).reshape(KP, 2, P, N).transpose(2, 0, 1, 3)
    ).astype(ml_dtypes.float8_e4m3)


def _chunk16(a):
    # [C, N] f32 -> [128, KI, N] fp16
    Cr, N = a.shape
    return np.ascontiguousarray(
        a.reshape(KI, P, N).transpose(1, 0, 2)).astype(np.float16)


def prepare(inputs):
    x = np.asarray(inputs["x"], np.float32)
    c = np.asarray(inputs["c"], np.float32)
    attn_mask = np.asarray(inputs["attn_mask"])
    padding_mask = np.asarray(inputs["padding_mask"])
    W = {n: np.asarray(inputs["W" + n], np.float32)
         for n in ["q", "k", "v", "kc", "vc", "g1", "g2", "p"]}
    bvec = {n: np.asarray(inputs["b" + n], np.float32)
            for n in ["q", "k", "v", "kc", "vc", "g1", "g2", "p"]}
    has_b = {n: bool(np.any(bvec[n] != 0)) for n in bvec}

    mask2 = np.asarray(attn_mask).reshape(T, T)
    order, slots, group_specs, kv_tiles = _plan(mask2)
    npart = sum(g[1] for g in group_specs if g[2] is not None)
    pslots = slots[:npart]

    nc = build_program(tuple(slots), tuple(group_specs), kv_tiles, has_b)

    KV = kv_tiles * KT
    w8 = {n: _dr8(W[n], WS) for n in ["q", "k", "kc"]}
    g12p = _chunk16(np.concatenate([W["g1"], W["g2"], W["p"]], axis=1))

    in_maps = []
    for core in range(8):
        b, j = divmod(core, 4)
        rows = core_rows(core, order)
        xT = x[b].T.astype(np.float32)            # [C, T]
        xq = np.ascontiguousarray(xT[:, rows])
        cTf = np.zeros((C, MP), np.float32)
        cTf[:, :M] = c[b].T
        pad = np.zeros((P, 1), np.float32)
        pad[:M, 0] = np.where(padding_mask[b] != 0, 0.0, NEG)
        a8 = np.concatenate([_dr8(xq), w8["q"], _dr8(cTf), w8["kc"]], axis=3)
        b8 = np.concatenate([w8["k"], _dr8(xT[:, :KV])], axis=3)
        xv = np.concatenate([_chunk16(W["v"]), _chunk16(xT[:, :KV])], axis=2)
        cv = np.concatenate([_chunk16(cTf), _chunk16(W["vc"])], axis=2)
        im = {"a8": a8, "b8": b8, "xv": xv, "cv": cv, "g12p": g12p,
              "padb": pad}
        if npart:
            mm = np.zeros((P, npart * QP), np.float16)
            for nslot, (cc, s) in enumerate(pslots):
                qrows = rows[cc * QP:(cc + 1) * QP]
                blk = mask2[qrows][:, s * KT:(s + 1) * KT]  # [QP, KT]
                mm[:, nslot * QP:(nslot + 1) * QP] = np.where(blk.T, 1.0, 0.0)
            im["maskm"] = mm
        for n in ["q", "k", "kc", "g1", "g2"]:
            if has_b[n]:
                v = bvec[n] * (1.0 / np.sqrt(D) if n == "q" else 1.0)
                im["b" + n] = np.ascontiguousarray(
                    v.reshape(PAIRS, P).T).astype(np.float32)
        for n in ["v", "vc", "p"]:
            if has_b[n]:
                im["b" + n] = bvec[n].reshape(1, C).astype(np.float16)
        in_maps.append(im)
    return nc, in_maps


def kernel(**inputs):
    nc, in_maps = prepare(inputs)
    res = bass_utils.run_bass_kernel_spmd(nc, in_maps, core_ids=list(range(8)))
    mask2 = np.asarray(inputs["attn_mask"]).reshape(T, T)
    order = _plan(mask2)[0]
    out = np.empty((B, T, C), np.float32)
    for core in range(8):
        b = core // 4
        rows = core_rows(core, order)
        out[b, rows] = res.results[core]["out"].astype(np.float32)
    return out
